# revision 1
# baseline (speedup 1.0000x reference)
import sys
import os
sys.path.insert(0, '/opt/trn_rl_repo')
import numpy as np

N_CORES = 8
CHUNK = 25000      # max rows per gather-source chunk (int16 idx limit 32767)
WBLK = 64          # one-hot window width / dst 64-block
SWBLK = 512        # psum superwindow (one PSUM bank of fp32)
GB = 8             # tiles per gather batch (1024 idx = SWDGE ring limit per instruction)
CBLK = 32          # tiles per compare batch (one DVE is_equal instruction)
DLG = 128          # tiles per dstloc DMA (must be multiple of CBLK)
IDXG = 8           # gather batches per idx DMA slab


def _ceil(a, b):
    return -(-a // b)


def _wrap_idx_batch(arr):
    """[GB*128] int16 -> [128, GB*8] staged layout: idx i at (i%16, i//16), replicated x8."""
    n = arr.shape[0]
    w16 = arr.reshape(n // 16, 16).T          # [16, n//16]
    return np.tile(w16, (8, 1))               # [128, n//16]


def _build_direction(src, dst, S, n_src_rows):
    """Uniform cross-core tile structure for one edge direction.

    Edges are grouped per core (dst // S), bucketed by (dst 64-block m, src chunk k),
    each bucket padded to a common (max-over-cores) slot count of 128-edge tiles.

    Returns (meta, staged) where staged arrays are per-core.
    """
    E = src.shape[0]
    nch = _ceil(n_src_rows, CHUNK)
    nblk64 = _ceil(S, WBLK)
    nblk512 = _ceil(S, SWBLK)
    nbuckets = nblk64 * nch

    core = dst // S
    dst_rel = dst - core * S
    k_arr = src // CHUNK
    src_loc = (src - k_arr * CHUNK).astype(np.int16)
    m_arr = dst_rel // WBLK
    bucket = m_arr * nch + k_arr

    # counts per (core, bucket)
    flat = core.astype(np.int64) * nbuckets + bucket
    counts = np.bincount(flat, minlength=N_CORES * nbuckets).reshape(N_CORES, nbuckets)
    slots = _ceil(counts, 128).max(axis=0)    # [nbuckets] tiles per bucket (0 if empty everywhere)

    ntiles = int(slots.sum())
    bucket_tile_start = np.zeros(nbuckets + 1, np.int64)
    np.cumsum(slots, out=bucket_tile_start[1:])

    # per-tile structure (same on every core)
    tile_bucket = np.repeat(np.arange(nbuckets), slots)
    tile_m = tile_bucket // nch
    tile_chunk = (tile_bucket % nch).astype(np.int64)
    tile_j = tile_m // (SWBLK // WBLK)                       # block512 id
    tile_off = (tile_m % (SWBLK // WBLK)) * WBLK             # psum free offset

    # block512 -> contiguous tile id range (tiles are m-major ordered)
    blk_start = np.searchsorted(tile_j, np.arange(nblk512), side='left')
    blk_end = np.searchsorted(tile_j, np.arange(nblk512), side='right')

    # per-chunk tile lists (global order) and gather batch assignment
    chunk_tiles = [np.nonzero(tile_chunk == k)[0] for k in range(nch)]
    nb_per_chunk = [_ceil(max(len(ct), 1), GB) for ct in chunk_tiles]
    tile_gb = np.zeros(ntiles, np.int64)      # batch index within chunk
    tile_gslot = np.zeros(ntiles, np.int64)   # slot within batch
    for k in range(nch):
        ct = chunk_tiles[k]
        pos = np.arange(len(ct))
        tile_gb[ct] = pos // GB
        tile_gslot[ct] = pos % GB

    # cumulative batch base per chunk (for idx staging offsets)
    batch_base = np.zeros(nch + 1, np.int64)
    np.cumsum(nb_per_chunk, out=batch_base[1:])
    NB = int(batch_base[-1])

    ntiles_pad = _ceil(max(ntiles, 1), DLG) * DLG

    # per-core staged arrays
    idx_staged = []
    dstloc_staged = []
    invrep = []
    order = np.lexsort((bucket, core))
    flat_sorted = flat[order]
    # rank of each edge within its (core,bucket) group
    grp_start = np.searchsorted(flat_sorted, flat_sorted, side='left')
    rank = np.arange(E) - grp_start

    for c in range(N_CORES):
        sel = order[core[order] == c]
        b_sel = bucket[sel]
        r_sel = rank[core[order] == c]
        gtile = bucket_tile_start[b_sel] + r_sel // 128
        lane = r_sel % 128

        idx_flat = np.zeros(ntiles * 128, np.int16)
        idx_flat[gtile * 128 + lane] = src_loc[sel]
        dl = np.full((128, ntiles_pad), -1.0, np.float32)
        dl[lane, gtile] = (dst_rel[sel] - tile_m[gtile] * WBLK).astype(np.float32)

        # idx staging: per chunk, batches of GB tiles, wrapped+replicated
        idx_cols = np.zeros((128, NB * GB * 8), np.int16)
        per_tile = idx_flat.reshape(ntiles, 128)
        for k in range(nch):
            ct = chunk_tiles[k]
            arr = np.zeros(nb_per_chunk[k] * GB * 128, np.int16)
            if len(ct):
                arr[:len(ct) * 128] = per_tile[ct].reshape(-1)
            for b in range(nb_per_chunk[k]):
                colw = GB * 8
                c0 = (batch_base[k] + b) * colw
                idx_cols[:, c0:c0 + colw] = _wrap_idx_batch(arr[b * GB * 128:(b + 1) * GB * 128])

        idx_staged.append(idx_cols)
        dstloc_staged.append(dl)

        cnt = np.bincount(dst_rel[core == c], minlength=S).astype(np.float32)
        inv = 1.0 / np.maximum(cnt, 1.0)
        invrep.append(np.tile(inv[None, :], (64, 1)).astype(np.float32))

    meta = dict(S=S, nch=nch, nblk512=nblk512, ntiles=ntiles, ntiles_pad=ntiles_pad,
                NB=NB, batch_base=batch_base, nb_per_chunk=nb_per_chunk,
                tile_chunk=tile_chunk, tile_off=tile_off, tile_gb=tile_gb,
                tile_gslot=tile_gslot, blk_start=blk_start, blk_end=blk_end)
    return meta, idx_staged, dstloc_staged, invrep


def kernel(author_features, edge_author, edge_paper, paper_emb, Wproj, bproj,
           W1l_ap, b1_ap, W1r_ap, W1l_pa, b1_pa, W1r_pa,
           W2l_ap, b2_ap, W2r_ap, W2l_pa, b2_pa, W2r_pa,
           Wl1, bl1, Wl2, bl2):
    import concourse.bass as bass
    import concourse.tile as tile
    from concourse import bacc, mybir
    from concourse.bass_utils import run_bass_kernel_spmd
    from concourse.library_config import mlp

    f32 = mybir.dt.float32
    i16 = mybir.dt.int16
    i32 = mybir.dt.int32
    AF = mybir.ActivationFunctionType

    author_features = np.asarray(author_features, np.float32)
    paper_emb = np.asarray(paper_emb, np.float32)
    edge_author = np.asarray(edge_author, np.int64)
    edge_paper = np.asarray(edge_paper, np.int64)

    NA, FIN = author_features.shape
    NP_, H = paper_emb.shape
    assert H == 64 and FIN == 128

    SA = _ceil(NA, N_CORES)
    SP = _ceil(NP_, N_CORES)

    # ---- host prep ----
    mAP, idxAP, dlAP, invP = _build_direction(edge_author, edge_paper, SP, NA)
    mPA, idxPA, dlPA, invA = _build_direction(edge_paper, edge_author, SA, NP_)
    nchA = mAP['nch']   # author-table chunks (AP convs gather authors)
    nchP = mPA['nch']   # paper-table chunks

    # per-core author-feature shard, transposed; padded to SA
    afT_cores = []
    for c in range(N_CORES):
        blk = author_features[c * SA: (c + 1) * SA]
        pad = np.zeros((SA, FIN), np.float32)
        pad[:blk.shape[0]] = blk
        afT_cores.append(np.ascontiguousarray(pad.T))
    # paper x_dst with ones row, per-core shard [65, SP]
    pT65_cores = []
    for c in range(N_CORES):
        blk = paper_emb[c * SP: (c + 1) * SP]
        t = np.zeros((65, SP), np.float32)
        t[:64, :blk.shape[0]] = blk.T
        t[64, :] = 1.0
        pT65_cores.append(t)
    p_rm = np.ascontiguousarray(paper_emb.astype(np.float32))
    ones_row = np.ones((1, max(SA, SP)), np.float32)
    iota_stage = np.tile(np.arange(WBLK, dtype=np.float32)[None, :], (128, 1))
    bprojc = np.asarray(bproj, np.float32).reshape(64, 1)
    bproj_rep = np.tile(np.asarray(bproj, np.float32).reshape(1, 64), (128, 1))

    W1r_b_ap = np.vstack([np.asarray(W1r_ap, np.float32), np.asarray(b1_ap, np.float32)[None]])
    W1r_b_pa = np.vstack([np.asarray(W1r_pa, np.float32), np.asarray(b1_pa, np.float32)[None]])
    W2r_b_ap = np.vstack([np.asarray(W2r_ap, np.float32), np.asarray(b2_ap, np.float32)[None]])
    Wl2c = np.asarray(Wl2, np.float32).reshape(64, 1)
    bl1c = np.asarray(bl1, np.float32).reshape(64, 1)
    bl2c = np.asarray(bl2, np.float32).reshape(1, 1)

    # ---- build program ----
    nc = bacc.Bacc("TRN2", target_bir_lowering=False, debug=False, num_devices=N_CORES)

    afT_h = nc.dram_tensor("afT", [128, SA], f32, kind="ExternalInput")
    p_rm_h = nc.dram_tensor("p_rm", [NP_, 64], f32, kind="ExternalInput")
    pT65_h = nc.dram_tensor("pT65", [65, SP], f32, kind="ExternalInput")
    invP_h = nc.dram_tensor("invP", [64, SP], f32, kind="ExternalInput")
    invA_h = nc.dram_tensor("invA", [64, SA], f32, kind="ExternalInput")
    idxAP_h = nc.dram_tensor("idxAP", list(idxAP[0].shape), i16, kind="ExternalInput")
    dlAP_h = nc.dram_tensor("dlAP", list(dlAP[0].shape), f32, kind="ExternalInput")
    idxPA_h = nc.dram_tensor("idxPA", list(idxPA[0].shape), i16, kind="ExternalInput")
    dlPA_h = nc.dram_tensor("dlPA", list(dlPA[0].shape), f32, kind="ExternalInput")
    ones_h = nc.dram_tensor("ones", [1, max(SA, SP)], f32, kind="ExternalInput")
    iota_h = nc.dram_tensor("iotaf", [128, WBLK], f32, kind="ExternalInput")
    w_names = ["Wproj", "W1l_ap", "W1rb_ap", "W1l_pa", "W1rb_pa", "W2l_ap", "W2rb_ap",
               "Wl1", "bl1", "Wl2", "bl2", "bprojc", "bprojrep"]
    w_vals = [np.asarray(Wproj, np.float32), np.asarray(W1l_ap, np.float32), W1r_b_ap,
              np.asarray(W1l_pa, np.float32), W1r_b_pa, np.asarray(W2l_ap, np.float32),
              W2r_b_ap, np.asarray(Wl1, np.float32), bl1c, Wl2c, bl2c, bprojc, bproj_rep]
    w_handles = {n: nc.dram_tensor(n, list(v.shape), f32, kind="ExternalInput")
                 for n, v in zip(w_names, w_vals)}
    out_h = nc.dram_tensor("out", [1, SP], f32, kind="ExternalOutput")

    ag_a_in = nc.dram_tensor("ag_a_in", [SA, 64], f32)
    a_full = nc.dram_tensor("a_full", [SA * N_CORES, 64], f32, addr_space="Shared")
    ag_a1_in = nc.dram_tensor("ag_a1_in", [SA, 64], f32)
    a1_full = nc.dram_tensor("a1_full", [SA * N_CORES, 64], f32, addr_space="Shared")
    p1T_h = nc.dram_tensor("p1T", [65, SP], f32)
    aT65_h = nc.dram_tensor("aT65", [65, SA], f32)

    rg = [list(range(N_CORES))]

    STAGE = int(os.environ.get("KERNEL_STAGE", "6"))
    with tile.TileContext(nc) as tc:
        import contextlib
        with contextlib.ExitStack() as ctx:
            const = ctx.enter_context(tc.tile_pool(name="const", bufs=1))
            msg_p = ctx.enter_context(tc.tile_pool(name="msg", bufs=12))
            idx_p = ctx.enter_context(tc.tile_pool(name="idx", bufs=8))
            oh_p = ctx.enter_context(tc.tile_pool(name="oh", bufs=4))
            dl_p = ctx.enter_context(tc.tile_pool(name="dl", bufs=3))
            inv_p = ctx.enter_context(tc.tile_pool(name="inv", bufs=3))
            mean_p = ctx.enter_context(tc.tile_pool(name="mean", bufs=3))
            x65_p = ctx.enter_context(tc.tile_pool(name="x65", bufs=3))
            sml_p = ctx.enter_context(tc.tile_pool(name="sml", bufs=4))
            big_p = ctx.enter_context(tc.tile_pool(name="big", bufs=2))
            aft_p = ctx.enter_context(tc.tile_pool(name="aft", bufs=2))
            ps0_p = ctx.enter_context(tc.tile_pool(name="ps0", bufs=2, space="PSUM"))
            psE_p = ctx.enter_context(tc.tile_pool(name="psE", bufs=3, space="PSUM"))
            psA_p = ctx.enter_context(tc.tile_pool(name="psA", bufs=2, space="PSUM"))
            psO_p = ctx.enter_context(tc.tile_pool(name="psO", bufs=1, space="PSUM"))

            nc.gpsimd.load_library(mlp)
            nidreg = nc.gpsimd.to_reg(GB * 128)

            wt = {}
            for n, v in zip(w_names, w_vals):
                t = const.tile(list(v.shape), f32, tag=f"w_{n}")
                nc.sync.dma_start(t[:], w_handles[n][:])
                wt[n] = t

            iota_f = const.tile([128, WBLK], f32, tag="iota_f")
            nc.sync.dma_start(iota_f[:], iota_h[:])
            zc = const.tile([1, 64], f32, tag="zc")
            nc.vector.memset(zc[:], 0.0)
            zr = const.tile([1, SWBLK], f32, tag="zr")
            nc.vector.memset(zr[:], 0.0)

            ones_sb = const.tile([1, max(SA, SP)], f32, tag="ones_sb")
            nc.sync.dma_start(ones_sb[:], ones_h[:])
            nc.sync.dma_start(aT65_h[64:65, :], ones_sb[:, :SA])
            nc.sync.dma_start(p1T_h[64:65, :], ones_sb[:, :SP])

            # ---- projection: a = af @ Wproj (+0 bias) ----
            for c0 in range(0, SA, SWBLK):
                cw = min(SWBLK, SA - c0)
                afT_t = aft_p.tile([128, SWBLK], f32)
                nc.sync.dma_start(afT_t[:, :cw], afT_h[:, c0:c0 + cw])
                ps = psE_p.tile([64, SWBLK], f32, tag="psE")
                nc.tensor.matmul(ps[:, :cw], wt["Wproj"][:], afT_t[:, :cw], start=True, stop=True)
                aTw = big_p.tile([64, SWBLK], f32, tag="o")
                nc.scalar.activation(aTw[:, :cw], ps[:, :cw], AF.Identity,
                                     bias=wt["bprojc"][:])
                nc.sync.dma_start(aT65_h[0:64, c0:c0 + cw], aTw[:, :cw])
                for s in range(0, cw, 128):
                    ws = min(128, cw - s)
                    psr = psA_p.tile([128, 64], f32, tag="psA")
                    nc.tensor.matmul(psr[:ws, :], afT_t[:, s:s + ws], wt["Wproj"][:],
                                     start=True, stop=True)
                    art = sml_p.tile([128, 64], f32)
                    nc.vector.tensor_tensor(out=art[:ws, :], in0=psr[:ws, :],
                                            in1=wt["bprojrep"][:ws, :],
                                            op=mybir.AluOpType.add)
                    nc.sync.dma_start(ag_a_in[c0 + s:c0 + s + ws, :], art[:ws, :])


            def emit_conv(meta, idx_h, dl_h, inv_h, src_tab, epilogue, after_blk0=None):
                SUB = os.environ.get("KERNEL_SUB", "e")
                dmaeng = nc.sync if int(os.environ.get("KERNEL_SYNCDMA", "0")) else nc.scalar
                S, nch = meta['S'], meta['nch']
                qn = [0]
                gb_cur = [-1] * nch
                ig_cur = [-1] * nch
                ig_t = [None] * nch
                msg_t = [None] * nch
                cb_cur, oh_t = -1, None
                dl_cur, dl_t = -1, None
                for j in range(meta['nblk512']):
                    c0 = j * SWBLK
                    C = min(SWBLK, S - c0)
                    t0, t1 = int(meta['blk_start'][j]), int(meta['blk_end'][j])
                    ps0 = ps0_p.tile([64, SWBLK], f32, tag="ps0")
                    nc.tensor.matmul(ps0[:], zc[:], zr[:], start=True, stop=False)
                    for t in range(t0, t1):
                        k = int(meta['tile_chunk'][t])
                        gb = int(meta['tile_gb'][t])
                        if gb != gb_cur[k]:
                            gb_cur[k] = gb
                            ig = gb // IDXG
                            if ig != ig_cur[k]:
                                ig_cur[k] = ig
                                ig_t[k] = idx_p.tile([128, IDXG * GB * 8], i16,
                                                     tag="idx", name=f"idx_{k}_{ig}")
                                icol = (int(meta['batch_base'][k]) + ig * IDXG) * GB * 8
                                nb_k = int(meta['nb_per_chunk'][k])
                                nld = min(IDXG, nb_k - ig * IDXG) * GB * 8
                                nc.sync.dma_start(ig_t[k][:, :nld],
                                                  idx_h[:, icol:icol + nld])
                            msg_t[k] = msg_p.tile([128, GB, 64], f32, tag="msg",
                                                  name=f"msg_{k}_{gb}")
                            goff = (gb % IDXG) * GB * 8
                            kbase = k * CHUNK
                            kend = min(kbase + CHUNK, src_tab.shape[0])
                            nc.gpsimd.dma_gather(msg_t[k][:], src_tab[kbase:kend, :],
                                                 ig_t[k][:, goff:goff + GB * 8],
                                                 GB * 128, nidreg, 64)
                            qn[0] += 1
                        if SUB == "a":
                            continue
                        cb = t // CBLK
                        if cb != cb_cur:
                            cb_cur = cb
                            dg = t // DLG
                            if dg != dl_cur:
                                dl_cur = dg
                                dl_t = dl_p.tile([128, DLG], f32)
                                dmaeng.dma_start(dl_t[:], dl_h[:, dg * DLG:(dg + 1) * DLG])
                            oh_t = oh_p.tile([128, CBLK, WBLK], f32)
                            dcol = (cb * CBLK) % DLG
                            in0 = dl_t[:, dcol:dcol + CBLK].to_broadcast([128, CBLK, WBLK])
                            _i = iota_f[:]
                            in1 = bass.AP(_i.tensor, _i.offset,
                                          [list(_i.ap[0]), [0, CBLK], list(_i.ap[1])])
                            nc.vector.tensor_tensor(out=oh_t[:], in0=in0, in1=in1,
                                                    op=mybir.AluOpType.is_equal)
                        if SUB == "b":
                            continue
                        off = int(meta['tile_off'][t])
                        nc.tensor.matmul(ps0[:, off:off + WBLK],
                                         msg_t[k][:, int(meta['tile_gslot'][t]), :],
                                         oh_t[:, t % CBLK, :],
                                         start=False, stop=False)
                    nc.tensor.matmul(ps0[:], zc[:], zr[:], start=False, stop=True)
                    if j == 0 and after_blk0 is not None:
                        after_blk0()
                    if SUB in ("a", "b", "c"):
                        continue
                    inv_t = inv_p.tile([64, SWBLK], f32)
                    dmaeng.dma_start(inv_t[:, :C], inv_h[:, c0:c0 + C])
                    meanT = mean_p.tile([64, SWBLK], f32)
                    nc.vector.tensor_tensor(out=meanT[:, :C], in0=ps0[:, :C],
                                            in1=inv_t[:, :C], op=mybir.AluOpType.mult)
                    if SUB == "d":
                        continue
                    epilogue(j, c0, C, meanT)

            # ---- conv1 PA: authors out, row-major -> ag_a1_in ----
            def epi_pa(j, c0, C, meanT):
                aT_t = x65_p.tile([65, SWBLK], f32, tag="pT_t")
                nc.scalar.dma_start(aT_t[:, :C], aT65_h[:, c0:c0 + C])
                for s in range(0, C, 128):
                    ws = min(128, C - s)
                    psA = psA_p.tile([128, 64], f32, tag="psA")
                    nc.tensor.matmul(psA[:ws, :], meanT[:, s:s + ws], wt["W1l_pa"][:],
                                     start=True, stop=False)
                    nc.tensor.matmul(psA[:ws, :], aT_t[:, s:s + ws], wt["W1rb_pa"][:],
                                     start=False, stop=True)
                    o = sml_p.tile([128, 64], f32)
                    nc.scalar.activation(o[:ws, :], psA[:ws, :], AF.Relu)
                    nc.sync.dma_start(ag_a1_in[c0 + s:c0 + s + ws, :], o[:ws, :])

            def trig_ag_a():
                if STAGE >= 2:
                    nc.gpsimd.collective_compute("AllGather", mybir.AluOpType.bypass,
                                                 replica_groups=rg, ins=[ag_a_in[:]],
                                                 outs=[a_full[:]])

            if STAGE >= 3:
                emit_conv(mPA, idxPA_h, dlPA_h, invA_h, p_rm_h, epi_pa,
                          after_blk0=trig_ag_a)
            else:
                trig_ag_a()

            def trig_ag_a1():
                if STAGE >= 4:
                    nc.gpsimd.collective_compute("AllGather", mybir.AluOpType.bypass,
                                                 replica_groups=rg, ins=[ag_a1_in[:]],
                                                 outs=[a1_full[:]])

            # ---- conv1 AP: papers out, feature-major -> p1T_h ----
            def epi_c1(j, c0, C, meanT):
                pT_t = x65_p.tile([65, SWBLK], f32, tag="pT_t")
                nc.sync.dma_start(pT_t[:, :C], pT65_h[:, c0:c0 + C])
                psE = psE_p.tile([64, SWBLK], f32, tag="psE")
                nc.tensor.matmul(psE[:, :C], wt["W1l_ap"][:], meanT[:, :C],
                                 start=True, stop=False)
                nc.tensor.matmul(psE[:, :C], wt["W1rb_ap"][:], pT_t[:, :C],
                                 start=False, stop=True)
                o = big_p.tile([64, SWBLK], f32)
                nc.scalar.activation(o[:, :C], psE[:, :C], AF.Relu)
                nc.sync.dma_start(p1T_h[0:64, c0:c0 + C], o[:, :C])

            if STAGE >= 5:
                emit_conv(mAP, idxAP_h, dlAP_h, invP_h, a_full, epi_c1,
                          after_blk0=trig_ag_a1)
            else:
                trig_ag_a1()

            # ---- conv2 AP + head ----
            def epi_c2(j, c0, C, meanT):
                p1_t = x65_p.tile([65, SWBLK], f32, tag="pT_t")
                nc.sync.dma_start(p1_t[:, :C], p1T_h[:, c0:c0 + C])
                psE = psE_p.tile([64, SWBLK], f32, tag="psE")
                nc.tensor.matmul(psE[:, :C], wt["W2l_ap"][:], meanT[:, :C],
                                 start=True, stop=False)
                nc.tensor.matmul(psE[:, :C], wt["W2rb_ap"][:], p1_t[:, :C],
                                 start=False, stop=True)
                p2 = big_p.tile([64, SWBLK], f32)
                nc.scalar.activation(p2[:, :C], psE[:, :C], AF.Relu)
                psH = psE_p.tile([64, SWBLK], f32, tag="psE")
                nc.tensor.matmul(psH[:, :C], wt["Wl1"][:], p2[:, :C], start=True, stop=True)
                h = big_p.tile([64, SWBLK], f32)
                nc.scalar.activation(h[:, :C], psH[:, :C], AF.Relu, bias=wt["bl1"][:])
                psO = psO_p.tile([1, SWBLK], f32, tag="psO")
                nc.tensor.matmul(psO[:, :C], wt["Wl2"][:], h[:, :C], start=True, stop=True)
                o = sml_p.tile([1, SWBLK], f32, tag="outrow")
                nc.scalar.activation(o[:, :C], psO[:, :C], AF.Identity, bias=wt["bl2"][:])
                nc.sync.dma_start(out_h[:, c0:c0 + C], o[:, :C])

            if STAGE >= 6:
                emit_conv(mAP, idxAP_h, dlAP_h, invP_h, a1_full, epi_c2)

    nc.compile()

    if int(os.environ.get("KERNEL_TLSIM", "1")):
        try:
            from concourse.timeline_sim import TimelineSim
            _t = TimelineSim(nc)
            kernel.modeled_time_ns = _t.simulate()
            print(f"[kernel] TimelineSim modeled core time: "
                  f"{kernel.modeled_time_ns / 1e3:.1f} us")
        except Exception as e:
            print(f"[kernel] TimelineSim failed: {e}")
            kernel.modeled_time_ns = None

    globals()["_last_nc"] = nc
    if int(os.environ.get("KERNEL_BUILD_ONLY", "0")):
        raise SystemExit(0)

    in_maps = []
    for c in range(N_CORES):
        m = {"afT": afT_cores[c], "p_rm": p_rm, "pT65": pT65_cores[c],
             "invP": invP[c], "invA": invA[c],
             "idxAP": idxAP[c], "dlAP": dlAP[c],
             "idxPA": idxPA[c], "dlPA": dlPA[c],
             "ones": ones_row,
             "iotaf": iota_stage}
        for n, v in zip(w_names, w_vals):
            m[n] = v
        in_maps.append(m)

    if int(os.environ.get("KERNEL_SIM", "0")):
        from concourse import bass_interp
        sim = bass_interp.MultiCoreSim(nc, N_CORES)
        for c in range(N_CORES):
            for n, v in in_maps[c].items():
                sim.cores[c].tensor(n)[:] = v
        sim.simulate()
        results = [{"out": np.array(sim.cores[c].tensor("out"))} for c in range(N_CORES)]
    else:
        trace = bool(int(os.environ.get("KERNEL_TRACE", "0")))
        res = run_bass_kernel_spmd(nc, in_maps, core_ids=list(range(N_CORES)), trace=trace)
        if trace:
            kernel.last_exec_time_ns = res.exec_time_ns
            kernel.last_results = res
        results = res.results

    out = np.concatenate([results[c]["out"][0] for c in range(N_CORES)])[:NP_]
    return out.reshape(NP_, 1).astype(np.float32)



# revision 36
# speedup vs baseline: 1.4624x; 1.4624x over previous
import sys
import os
sys.path.insert(0, '/opt/trn_rl_repo')
import numpy as np
import ml_dtypes

BF = ml_dtypes.bfloat16

N_CORES = 8
GB = int(os.environ.get("KERNEL_GB", "8"))    # tiles per gather batch
MSGB = int(os.environ.get("KERNEL_MSGB", "16"))  # msg pool buffers
WBLK = 64          # one-hot window width
SWBLK = 512        # psum superwindow (one PSUM bank of fp32)
CBLK = 32          # tiles per DVE is_equal batch
DLG = 128          # tiles per dstloc DMA (multiple of CBLK)
IDXG = 4           # gather batches per idx DMA slab
SCRATCH = int(os.environ.get("KERNEL_SCRATCH", "98304"))    # SWDGE ring bytes -> 4096 descriptors (2 batches in flight)

NA, NP_, FIN, H = 100000, 150000, 128, 64
SA, SP = NA // N_CORES, NP_ // N_CORES          # 12500, 18750
SPLIT = 6144                                    # catA rows/core (12 PA blocks)
SPLIT_B = SA - SPLIT                            # 6356
CATA_N = SPLIT * N_CORES                        # 49152
CATB_N = SPLIT_B * N_CORES                      # 50848
BOUND_PA = np.arange(0, NP_ + 24999, 25000)[:7]             # 6 chunks of 25000
BOUND_S0 = np.array([0, CATA_N // 2, CATA_N])               # 2 x 24576
BOUND_S1 = np.array([0, CATB_N // 2, CATB_N])               # 2 x 25424


def _ceil(a, b):
    return -(-a // b)


def _wrap_idx_batch(arr):
    """[GB*128] int16 -> [128, GB*8] staged layout: idx i at (i%16, i//16), x8."""
    n = arr.shape[0]
    w16 = arr.reshape(n // 16, 16).T
    return np.tile(w16, (8, 1))


def _build_pass(dst, src_gid, sel, S, bounds):
    """One gather/aggregate pass: edges [sel], dst sharded by S across cores,
    src gathered from a table addressed by src_gid with chunk boundaries."""
    idx_e = np.nonzero(sel)[0]
    d = dst[idx_e].astype(np.int64)
    s = src_gid[idx_e].astype(np.int64)
    core = d // S
    dst_rel = d - core * S
    nch = len(bounds) - 1
    k_arr = np.searchsorted(bounds, s, side='right') - 1
    src_loc = (s - bounds[k_arr]).astype(np.int16)
    nblk64 = _ceil(S, WBLK)
    nblk512 = _ceil(S, SWBLK)
    m_arr = dst_rel // WBLK
    bucket = m_arr * nch + k_arr
    nbuckets = nblk64 * nch

    flat = core * nbuckets + bucket
    counts = np.bincount(flat, minlength=N_CORES * nbuckets).reshape(N_CORES, nbuckets)
    slots = _ceil(counts, 128).max(axis=0)
    ntiles = int(slots.sum())
    bucket_tile_start = np.zeros(nbuckets + 1, np.int64)
    np.cumsum(slots, out=bucket_tile_start[1:])

    tile_bucket = np.repeat(np.arange(nbuckets), slots)
    tile_m = tile_bucket // nch
    tile_chunk = (tile_bucket % nch).astype(np.int64)
    tile_j = tile_m // (SWBLK // WBLK)
    tile_off = (tile_m % (SWBLK // WBLK)) * WBLK

    blk_start = np.searchsorted(tile_j, np.arange(nblk512), side='left')
    blk_end = np.searchsorted(tile_j, np.arange(nblk512), side='right')

    # per-window psum start/stop flags (window = (j, m) run of tiles)
    w_start = np.ones(ntiles, bool)
    w_stop = np.ones(ntiles, bool)
    if ntiles > 1:
        same = tile_m[1:] == tile_m[:-1]
        w_start[1:] = ~same
        w_stop[:-1] = ~same
    # windows with no tiles at all need explicit psum zeroing
    present = np.zeros(nblk64, bool)
    present[tile_m] = True
    empty_w = [[] for _ in range(nblk512)]
    for m in np.nonzero(~present)[0]:
        empty_w[m // (SWBLK // WBLK)].append(int((m % (SWBLK // WBLK)) * WBLK))

    # max fill (over cores) of each tile; only a bucket's last tile is partial
    tile_fill = np.full(ntiles, 128, np.int64)
    cmax = counts.max(axis=0)
    for b in np.nonzero(slots)[0]:
        last = bucket_tile_start[b + 1] - 1
        tile_fill[last] = min(max(int(cmax[b]) - 128 * (int(slots[b]) - 1), 1), 128)

    # gather batches per chunk
    chunk_tiles = [np.nonzero(tile_chunk == k)[0] for k in range(nch)]
    nb_per_chunk = [_ceil(len(ct), GB) if len(ct) else 0 for ct in chunk_tiles]
    tile_gb = np.zeros(ntiles, np.int64)
    tile_gslot = np.zeros(ntiles, np.int64)
    batch_sizes = []           # [k][b] -> real tiles in batch
    batch_nidx = []            # [k][b] -> num_idxs (last tile trimmed to 16)
    for k in range(nch):
        ct = chunk_tiles[k]
        pos = np.arange(len(ct))
        tile_gb[ct] = pos // GB
        tile_gslot[ct] = pos % GB
        bs = [min(GB, len(ct) - b * GB) for b in range(nb_per_chunk[k])]
        batch_sizes.append(bs)
        bn = []
        for b in range(nb_per_chunk[k]):
            tt = ct[b * GB:(b + 1) * GB]
            bn.append((len(tt) - 1) * 128 + _ceil(int(tile_fill[tt[-1]]), 16) * 16)
        batch_nidx.append(bn)
    batch_base = np.zeros(nch + 1, np.int64)
    np.cumsum(nb_per_chunk, out=batch_base[1:])
    NB = max(int(batch_base[-1]), 1)

    ntiles_pad = _ceil(max(ntiles, 1), DLG) * DLG

    order = np.lexsort((bucket, core))
    flat_sorted = flat[order]
    grp_start = np.searchsorted(flat_sorted, flat_sorted, side='left')
    rank = np.arange(len(idx_e)) - grp_start

    idx_staged, dl_staged = [], []
    core_ord = core[order]
    for c in range(N_CORES):
        selc = order[core_ord == c]
        r_sel = rank[core_ord == c]
        b_sel = bucket[selc]
        gtile = bucket_tile_start[b_sel] + r_sel // 128
        lane = r_sel % 128

        idx_flat = np.zeros(ntiles * 128, np.int16)
        idx_flat[gtile * 128 + lane] = src_loc[selc]
        dl = np.full((128, ntiles_pad), -1.0, BF)
        dl[lane, gtile] = (dst_rel[selc] - tile_m[gtile] * WBLK).astype(BF)

        idx_cols = np.zeros((128, NB * GB * 8), np.int16)
        per_tile = idx_flat.reshape(ntiles, 128)
        for k in range(nch):
            ct = chunk_tiles[k]
            for b in range(nb_per_chunk[k]):
                arr = np.zeros(GB * 128, np.int16)
                tt = ct[b * GB:(b + 1) * GB]
                arr[:len(tt) * 128] = per_tile[tt].reshape(-1)
                c0 = (batch_base[k] + b) * GB * 8
                idx_cols[:, c0:c0 + GB * 8] = _wrap_idx_batch(arr)
        idx_staged.append(idx_cols)
        dl_staged.append(dl)

    meta = dict(S=S, nch=nch, bounds=bounds, nblk512=nblk512, ntiles=ntiles,
                ntiles_pad=ntiles_pad, NB=NB, batch_base=batch_base,
                nb_per_chunk=nb_per_chunk, batch_sizes=batch_sizes,
                batch_nidx=batch_nidx,
                tile_chunk=tile_chunk, tile_off=tile_off, tile_gb=tile_gb,
                tile_gslot=tile_gslot, blk_start=blk_start, blk_end=blk_end,
                w_start=w_start, w_stop=w_stop, empty_w=empty_w)
    return meta, idx_staged, dl_staged


def kernel(author_features, edge_author, edge_paper, paper_emb, Wproj, bproj,
           W1l_ap, b1_ap, W1r_ap, W1l_pa, b1_pa, W1r_pa,
           W2l_ap, b2_ap, W2r_ap, W2l_pa, b2_pa, W2r_pa,
           Wl1, bl1, Wl2, bl2):
    import concourse.bass as bass
    import concourse.tile as tile
    from concourse import bacc, mybir
    from concourse.bass_utils import run_bass_kernel_spmd
    from concourse.library_config import mlp

    f32 = mybir.dt.float32
    bf16 = mybir.dt.bfloat16
    i16 = mybir.dt.int16
    AF = mybir.ActivationFunctionType

    af = np.asarray(author_features, np.float32)
    p = np.asarray(paper_emb, np.float32)
    ea = np.asarray(edge_author, np.int64)
    ep = np.asarray(edge_paper, np.int64)
    E = ea.shape[0]

    # ---- host prep: cat-table row mapping for authors ----
    c_a = ea // SA
    r_a = ea - c_a * SA
    gidA = np.where(r_a < SPLIT, c_a * SPLIT + r_a,
                    CATA_N + c_a * SPLIT_B + (r_a - SPLIT))
    selS0 = gidA < CATA_N

    mPA, idxPA, dlPA = _build_pass(ea, ep, np.ones(E, bool), SA, BOUND_PA)
    mS0, idxS0, dlS0 = _build_pass(ep, gidA, selS0, SP, BOUND_S0)
    mS1, idxS1, dlS1 = _build_pass(ep, gidA - CATA_N, ~selS0, SP, BOUND_S1)

    # ---- staged tables ----
    afT_cores = [np.ascontiguousarray(af[c * SA:(c + 1) * SA].T).astype(BF)
                 for c in range(N_CORES)]
    p_bf = np.zeros((NP_, 128), BF)
    p_bf[:, :64] = p.astype(BF)
    pT_cores = [np.ascontiguousarray(p[c * SP:(c + 1) * SP].T).astype(BF)
                for c in range(N_CORES)]

    invA_cores, invP_cores = [], []
    for c in range(N_CORES):
        cntA = np.bincount(ea[ea // SA == c] - c * SA, minlength=SA).astype(np.float32)
        cntP = np.bincount(ep[ep // SP == c] - c * SP, minlength=SP).astype(np.float32)
        invA_cores.append(np.tile((1.0 / np.maximum(cntA, 1.0))[None, :], (64, 1)).astype(BF))
        invP_cores.append(np.tile((1.0 / np.maximum(cntP, 1.0))[None, :], (128, 1)).astype(BF))

    iota_stage = np.tile(np.arange(WBLK, dtype=np.float32)[None, :], (128, 1)).astype(BF)
    ones_bf = np.ones((1, SA), BF)
    W1rb_pa = np.vstack([np.asarray(W1r_pa, np.float32),
                         np.asarray(b1_pa, np.float32)[None]])

    wb_names = ["Wproj", "W1l_ap", "W1r_ap", "W1l_pa", "W1rb_pa",
                "W2l_ap", "W2r_ap", "Wl1", "Wl2c"]
    wb_vals = [np.asarray(Wproj, np.float32).astype(BF),
               np.asarray(W1l_ap, np.float32).astype(BF),
               np.asarray(W1r_ap, np.float32).astype(BF),
               np.asarray(W1l_pa, np.float32).astype(BF),
               W1rb_pa.astype(BF),
               np.asarray(W2l_ap, np.float32).astype(BF),
               np.asarray(W2r_ap, np.float32).astype(BF),
               np.asarray(Wl1, np.float32).astype(BF),
               np.asarray(Wl2, np.float32).reshape(64, 1).astype(BF)]
    # cat 'a' half is stored WITHOUT bproj; fold W1l_ap^T @ bproj into b1
    b1_eff = (np.asarray(b1_ap, np.float32)
              + np.asarray(W1l_ap, np.float32).T @ np.asarray(bproj, np.float32))
    wf_names = ["bprojc", "b1c", "b2c", "bl1c", "bl2c"]
    wf_vals = [np.asarray(bproj, np.float32).reshape(64, 1),
               b1_eff.reshape(64, 1),
               np.asarray(b2_ap, np.float32).reshape(64, 1),
               np.asarray(bl1, np.float32).reshape(64, 1),
               np.asarray(bl2, np.float32).reshape(1, 1)]

    # ---- build program ----
    nc = bacc.Bacc("TRN2", target_bir_lowering=False, debug=False,
                   num_devices=N_CORES, dynamic_dma_scratch_size=SCRATCH)

    afT_h = nc.dram_tensor("afT", [128, SA], bf16, kind="ExternalInput")
    p_bf_h = nc.dram_tensor("p_bf", [NP_, 128], bf16, kind="ExternalInput")
    pT_h = nc.dram_tensor("pT", [64, SP], bf16, kind="ExternalInput")
    invA_h = nc.dram_tensor("invA", [64, SA], bf16, kind="ExternalInput")
    invP_h = nc.dram_tensor("invP", [128, SP], bf16, kind="ExternalInput")
    idxPA_h = nc.dram_tensor("idxPA", list(idxPA[0].shape), i16, kind="ExternalInput")
    dlPA_h = nc.dram_tensor("dlPA", list(dlPA[0].shape), bf16, kind="ExternalInput")
    idxS0_h = nc.dram_tensor("idxS0", list(idxS0[0].shape), i16, kind="ExternalInput")
    dlS0_h = nc.dram_tensor("dlS0", list(dlS0[0].shape), bf16, kind="ExternalInput")
    idxS1_h = nc.dram_tensor("idxS1", list(idxS1[0].shape), i16, kind="ExternalInput")
    dlS1_h = nc.dram_tensor("dlS1", list(dlS1[0].shape), bf16, kind="ExternalInput")
    ones_h = nc.dram_tensor("ones", [1, SA], bf16, kind="ExternalInput")
    iota_h = nc.dram_tensor("iotaf", [128, WBLK], bf16, kind="ExternalInput")
    wb_handles = {n: nc.dram_tensor(n, list(v.shape), bf16, kind="ExternalInput")
                  for n, v in zip(wb_names, wb_vals)}
    wf_handles = {n: nc.dram_tensor(n, list(v.shape), f32, kind="ExternalInput")
                  for n, v in zip(wf_names, wf_vals)}
    out_h = nc.dram_tensor("out", [1, SP], f32, kind="ExternalOutput")

    aT65_h = nc.dram_tensor("aT65", [65, SA], bf16)
    catA_in = nc.dram_tensor("catA_in", [SPLIT, 128], bf16)
    catB_in = nc.dram_tensor("catB_in", [SPLIT_B, 128], bf16)
    catA_full = nc.dram_tensor("catA_full", [CATA_N, 128], bf16, addr_space="Shared")
    catB_full = nc.dram_tensor("catB_full", [CATB_N, 128], bf16, addr_space="Shared")

    rg = [list(range(N_CORES))]
    nblkA = SA // SWBLK + (1 if SA % SWBLK else 0)      # 25
    nblkP = mS0['nblk512']                              # 37

    with tile.TileContext(nc) as tc:
        import contextlib
        with contextlib.ExitStack() as ctx:
            const = ctx.enter_context(tc.tile_pool(name="const", bufs=1))
            msg_p = ctx.enter_context(tc.tile_pool(name="msg", bufs=MSGB))
            idx_p = ctx.enter_context(tc.tile_pool(name="idx", bufs=14))
            oh_p = ctx.enter_context(tc.tile_pool(name="oh", bufs=3))
            dl_p = ctx.enter_context(tc.tile_pool(name="dl", bufs=3))
            inv_p = ctx.enter_context(tc.tile_pool(name="inv", bufs=3))
            mean_p = ctx.enter_context(tc.tile_pool(name="mean", bufs=3))
            sum_p = ctx.enter_context(tc.tile_pool(name="sum", bufs=2))
            x65_p = ctx.enter_context(tc.tile_pool(name="x65", bufs=3))
            big_p = ctx.enter_context(tc.tile_pool(name="big", bufs=4))
            sml_p = ctx.enter_context(tc.tile_pool(name="sml", bufs=4))
            aft_p = ctx.enter_context(tc.tile_pool(name="aft", bufs=2))
            outr_p = ctx.enter_context(tc.tile_pool(name="outr", bufs=2))
            ps0_p = ctx.enter_context(tc.tile_pool(name="ps0", bufs=int(os.environ.get("KERNEL_PS0B", "3")), space="PSUM"))
            psE_p = ctx.enter_context(tc.tile_pool(name="psE", bufs=2, space="PSUM"))
            psA_p = ctx.enter_context(tc.tile_pool(name="psA", bufs=2, space="PSUM"))
            psO_p = ctx.enter_context(tc.tile_pool(name="psO", bufs=1, space="PSUM"))

            nc.gpsimd.load_library(mlp)
            gcount = [0]
            _regs = {}

            def nreg(v):
                if v not in _regs:
                    _regs[v] = nc.gpsimd.to_reg(v)
                return _regs[v]

            nreg(GB * 128)

            wt = {}
            for n, v in zip(wb_names, wb_vals):
                t = const.tile(list(v.shape), bf16, tag=f"w_{n}")
                nc.sync.dma_start(t[:], wb_handles[n][:])
                wt[n] = t
            for n, v in zip(wf_names, wf_vals):
                t = const.tile(list(v.shape), f32, tag=f"w_{n}")
                nc.sync.dma_start(t[:], wf_handles[n][:])
                wt[n] = t

            iota_f = const.tile([128, WBLK], bf16, tag="iota_f")
            nc.sync.dma_start(iota_f[:], iota_h[:])
            zc1 = const.tile([1, 128], bf16, tag="zc1")
            nc.vector.memset(zc1[:], 0.0)
            zr64 = const.tile([1, WBLK], bf16, tag="zr64")
            nc.vector.memset(zr64[:], 0.0)
            nc.sync.dma_start(aT65_h[64:65, :], ones_h[:])
            partial = const.tile([128, nblkP * SWBLK], bf16, tag="partial")

            def cat_rows(r0, r1):
                """Map local author rows [r0,r1) to (tensor, row range)."""
                if r1 <= SPLIT:
                    return catA_in, r0, r1
                assert r0 >= SPLIT
                return catB_in, r0 - SPLIT, r1 - SPLIT

            # ---- projection block: a = af @ Wproj (+bproj only in aT65) ----
            def proj_block(b):
                c0 = b * SWBLK
                cw = min(SWBLK, SA - c0)
                afT_t = aft_p.tile([128, SWBLK], bf16, tag="aft", name=f"afT_{b}")
                nc.sync.dma_start(afT_t[:, :cw], afT_h[:, c0:c0 + cw])
                ps = psE_p.tile([64, SWBLK], f32, tag="psE")
                nc.tensor.matmul(ps[:, :cw], wt["Wproj"][:], afT_t[:, :cw],
                                 start=True, stop=True)
                aTw = big_p.tile([64, SWBLK], bf16, tag="big")
                nc.scalar.activation(aTw[:, :cw], ps[:, :cw], AF.Identity,
                                     bias=wt["bprojc"][:])
                nc.sync.dma_start(aT65_h[0:64, c0:c0 + cw], aTw[:, :cw])
                for s in range(0, cw, 128):
                    ws = min(128, cw - s)
                    psr = psA_p.tile([128, 64], f32, tag="psA")
                    nc.tensor.matmul(psr[:ws, :], afT_t[:, s:s + ws], wt["Wproj"][:],
                                     start=True, stop=True)
                    art = sml_p.tile([128, 64], bf16, tag="sml")
                    nc.scalar.activation(art[:ws, :], psr[:ws, :], AF.Identity)
                    tgt, t0r, t1r = cat_rows(c0 + s, c0 + s + ws)
                    nc.sync.dma_start(tgt[t0r:t1r, 0:64], art[:ws, :])

            proj_block(0)
            proj_block(1)

            # ---- generic gather/aggregate pass ----
            def emit_pass(meta, idx_h, dl_h, src_tab_fn, close_block, pname,
                          extra=None):
                nch = meta['nch']
                gb_cur = [-1] * nch
                ig_t = [dict() for _ in range(nch)]
                msg_t = [None] * nch
                cb_cur, oh_t = [-1], [None]
                dl_cur, dl_t = [-1], [None]

                def load_slab(k, ig):
                    t = idx_p.tile([128, IDXG * GB * 8], i16, tag="idx",
                                   name=f"idx{pname}_{k}_{ig}")
                    icol = (int(meta['batch_base'][k]) + ig * IDXG) * GB * 8
                    nb_k = int(meta['nb_per_chunk'][k])
                    nld = min(IDXG, nb_k - ig * IDXG) * GB * 8
                    nc.sync.dma_start(t[:, :nld], idx_h[:, icol:icol + nld])
                    ig_t[k][ig] = t
                for j in range(meta['nblk512']):
                    c0 = j * SWBLK
                    C = min(SWBLK, meta['S'] - c0)
                    ps0 = ps0_p.tile([128, SWBLK], f32, tag="ps0")
                    for off in meta['empty_w'][j]:
                        nc.tensor.matmul(ps0[:, off:off + WBLK], zc1[:], zr64[:],
                                         start=True, stop=True)
                    t0, t1 = int(meta['blk_start'][j]), int(meta['blk_end'][j])
                    for t in range(t0, t1):
                        k = int(meta['tile_chunk'][t])
                        gb = int(meta['tile_gb'][t])
                        if gb != gb_cur[k]:
                            gb_cur[k] = gb
                            ig = gb // IDXG
                            nb_k = int(meta['nb_per_chunk'][k])
                            if ig not in ig_t[k]:
                                load_slab(k, ig)
                            if gb % IDXG == 0 and (ig + 1) * IDXG < nb_k \
                                    and (ig + 1) not in ig_t[k]:
                                load_slab(k, ig + 1)
                            if ig - 1 in ig_t[k]:
                                del ig_t[k][ig - 1]
                            msg_t[k] = msg_p.tile([128, GB, 128], bf16, tag="msg",
                                                  name=f"msg{pname}_{k}_{gb}")
                            goff = (gb % IDXG) * GB * 8
                            tab, kbase, kend = src_tab_fn(k)
                            nreal = meta['batch_sizes'][k][gb]
                            nidx = int(meta['batch_nidx'][k][gb])
                            _gi = nc.gpsimd.dma_gather(msg_t[k][:, :nreal, :],
                                                       tab[kbase:kend, :],
                                                       ig_t[k][ig][:, goff:goff + GB * 8],
                                                       nidx, nreg(nidx), 128)
                            if os.environ.get("KERNEL_FOLLOW") and pname == "pa":
                                gcount[0] += 1
                                if 70 <= gcount[0] <= 78:
                                    tile.tile_follow(_gi, log_all_deps=True)
                        cb = t // CBLK
                        if cb != cb_cur[0]:
                            cb_cur[0] = cb
                            dg = t // DLG
                            if dg != dl_cur[0]:
                                dl_cur[0] = dg
                                dl_t[0] = dl_p.tile([128, DLG], bf16, tag="dl",
                                                    name=f"dl{pname}_{dg}")
                                nc.scalar.dma_start(dl_t[0][:],
                                                    dl_h[:, dg * DLG:(dg + 1) * DLG])
                            oh_t[0] = oh_p.tile([128, CBLK, WBLK], bf16, tag="oh",
                                                name=f"oh{pname}_{cb}")
                            dcol = (cb * CBLK) % DLG
                            in0 = dl_t[0][:, dcol:dcol + CBLK].to_broadcast(
                                [128, CBLK, WBLK])
                            _i = iota_f[:]
                            in1 = bass.AP(_i.tensor, _i.offset,
                                          [list(_i.ap[0]), [0, CBLK], list(_i.ap[1])])
                            nc.vector.tensor_tensor(out=oh_t[0][:], in0=in0, in1=in1,
                                                    op=mybir.AluOpType.is_equal)
                        off = int(meta['tile_off'][t])
                        nc.tensor.matmul(ps0[:, off:off + WBLK],
                                         msg_t[k][:, int(meta['tile_gslot'][t]), :],
                                         oh_t[0][:, t % CBLK, :],
                                         start=bool(meta['w_start'][t]),
                                         stop=bool(meta['w_stop'][t]))
                    close_block(j, c0, C, ps0)
                    if extra is not None:
                        extra(j)

            # ---- PA pass: papers -> authors, produces a1 into cat tables ----
            def close_pa(j, c0, C, ps0):
                inv_t = inv_p.tile([64, SWBLK], bf16, tag="inv64")
                nc.scalar.dma_start(inv_t[:, :C], invA_h[:, c0:c0 + C])
                meanT = mean_p.tile([64, SWBLK], bf16, tag="mean64")
                nc.vector.tensor_tensor(out=meanT[:, :C], in0=ps0[0:64, :C],
                                        in1=inv_t[:, :C], op=mybir.AluOpType.mult)
                aT_t = x65_p.tile([65, SWBLK], bf16, tag="x65")
                nc.scalar.dma_start(aT_t[:, :C], aT65_h[:, c0:c0 + C])
                for s in range(0, C, 128):
                    ws = min(128, C - s)
                    psA = psA_p.tile([128, 64], f32, tag="psA")
                    nc.tensor.matmul(psA[:ws, :], meanT[:, s:s + ws], wt["W1l_pa"][:],
                                     start=True, stop=False)
                    nc.tensor.matmul(psA[:ws, :], aT_t[:, s:s + ws], wt["W1rb_pa"][:],
                                     start=False, stop=True)
                    o = sml_p.tile([128, 64], bf16, tag="sml")
                    nc.scalar.activation(o[:ws, :], psA[:ws, :], AF.Relu)
                    tgt, t0r, t1r = cat_rows(c0 + s, c0 + s + ws)
                    nc.sync.dma_start(tgt[t0r:t1r, 64:128], o[:ws, :])
                if STAGE < 3:
                    return
                if j == SPLIT // SWBLK - 1:
                    nc.gpsimd.collective_compute("AllGather", mybir.AluOpType.bypass,
                                                 replica_groups=rg, ins=[catA_in[:]],
                                                 outs=[catA_full[:]])
                if j == nblkA - 1:
                    nc.gpsimd.collective_compute("AllGather", mybir.AluOpType.bypass,
                                                 replica_groups=rg, ins=[catB_in[:]],
                                                 outs=[catB_full[:]])

            STAGE = int(os.environ.get("KERNEL_STAGE", "5"))

            def pa_extra(j):
                if j + 2 < nblkA:
                    proj_block(j + 2)

            if STAGE >= 2:
                emit_pass(mPA, idxPA_h, dlPA_h,
                          lambda k: (p_bf_h, k * 25000, min((k + 1) * 25000, NP_)),
                          close_pa, "pa", extra=pa_extra)

            # ---- AP sweep 0: accumulate catA contributions into partials ----
            def close_s0(j, c0, C, ps0):
                nc.scalar.activation(partial[:, c0:c0 + C], ps0[:, :C], AF.Identity)

            if STAGE >= 4:
                emit_pass(mS0, idxS0_h, dlS0_h,
                          lambda k: (catA_full, int(BOUND_S0[k]), int(BOUND_S0[k + 1])),
                          close_s0, "s0")

            # ---- AP sweep 1 + fused conv1/conv2/head epilogue ----
            def close_s1(j, c0, C, ps0):
                sum_t = sum_p.tile([128, SWBLK], f32, tag="sum")
                nc.vector.tensor_tensor(out=sum_t[:, :C], in0=ps0[:, :C],
                                        in1=partial[:, c0:c0 + C],
                                        op=mybir.AluOpType.add)
                inv_t = inv_p.tile([128, SWBLK], bf16, tag="inv128")
                nc.scalar.dma_start(inv_t[:, :C], invP_h[:, c0:c0 + C])
                meanA = mean_p.tile([64, SWBLK], bf16, tag="meanA")
                nc.vector.tensor_tensor(out=meanA[:, :C], in0=sum_t[0:64, :C],
                                        in1=inv_t[0:64, :C], op=mybir.AluOpType.mult)
                meanB = mean_p.tile([64, SWBLK], bf16, tag="meanB")
                nc.vector.tensor_tensor(out=meanB[:, :C], in0=sum_t[64:128, :C],
                                        in1=inv_t[64:128, :C], op=mybir.AluOpType.mult)
                pT_t = x65_p.tile([64, SWBLK], bf16, tag="pTt")
                nc.sync.dma_start(pT_t[:, :C], pT_h[:, c0:c0 + C])
                psE = psE_p.tile([64, SWBLK], f32, tag="psE")
                nc.tensor.matmul(psE[:, :C], wt["W1l_ap"][:], meanA[:, :C],
                                 start=True, stop=False)
                nc.tensor.matmul(psE[:, :C], wt["W1r_ap"][:], pT_t[:, :C],
                                 start=False, stop=True)
                p1 = big_p.tile([64, SWBLK], bf16, tag="big")
                nc.scalar.activation(p1[:, :C], psE[:, :C], AF.Relu, bias=wt["b1c"][:])
                psE2 = psE_p.tile([64, SWBLK], f32, tag="psE")
                nc.tensor.matmul(psE2[:, :C], wt["W2l_ap"][:], meanB[:, :C],
                                 start=True, stop=False)
                nc.tensor.matmul(psE2[:, :C], wt["W2r_ap"][:], p1[:, :C],
                                 start=False, stop=True)
                p2 = big_p.tile([64, SWBLK], bf16, tag="big")
                nc.scalar.activation(p2[:, :C], psE2[:, :C], AF.Relu, bias=wt["b2c"][:])
                psH = psE_p.tile([64, SWBLK], f32, tag="psE")
                nc.tensor.matmul(psH[:, :C], wt["Wl1"][:], p2[:, :C],
                                 start=True, stop=True)
                h = big_p.tile([64, SWBLK], bf16, tag="big")
                nc.scalar.activation(h[:, :C], psH[:, :C], AF.Relu, bias=wt["bl1c"][:])
                psO = psO_p.tile([1, SWBLK], f32, tag="psO")
                nc.tensor.matmul(psO[:, :C], wt["Wl2c"][:], h[:, :C],
                                 start=True, stop=True)
                o = outr_p.tile([1, SWBLK], f32, tag="outrow")
                nc.scalar.activation(o[:, :C], psO[:, :C], AF.Identity,
                                     bias=wt["bl2c"][:])
                nc.sync.dma_start(out_h[:, c0:c0 + C], o[:, :C])

            if STAGE >= 5:
                emit_pass(mS1, idxS1_h, dlS1_h,
                          lambda k: (catB_full, int(BOUND_S1[k]), int(BOUND_S1[k + 1])),
                          close_s1, "s1")

    nc.compile()

    if int(os.environ.get("KERNEL_TLSIM", "1")):
        try:
            from concourse.timeline_sim import TimelineSim
            _t = TimelineSim(nc)
            kernel.modeled_time_ns = _t.simulate()
            print(f"[kernel] TimelineSim modeled core time: "
                  f"{kernel.modeled_time_ns / 1e3:.1f} us")
        except Exception as e:
            print(f"[kernel] TimelineSim failed: {e}")
            kernel.modeled_time_ns = None

    globals()["_last_nc"] = nc
    if int(os.environ.get("KERNEL_BUILD_ONLY", "0")):
        raise SystemExit(0)

    in_maps = []
    for c in range(N_CORES):
        m = {"afT": afT_cores[c], "p_bf": p_bf, "pT": pT_cores[c],
             "invA": invA_cores[c], "invP": invP_cores[c],
             "idxPA": idxPA[c], "dlPA": dlPA[c],
             "idxS0": idxS0[c], "dlS0": dlS0[c],
             "idxS1": idxS1[c], "dlS1": dlS1[c],
             "ones": ones_bf, "iotaf": iota_stage}
        for n, v in zip(wb_names, wb_vals):
            m[n] = v
        for n, v in zip(wf_names, wf_vals):
            m[n] = v
        in_maps.append(m)

    if int(os.environ.get("KERNEL_SIM", "0")):
        from concourse import bass_interp
        sim = bass_interp.MultiCoreSim(nc, N_CORES)
        for c in range(N_CORES):
            for n, v in in_maps[c].items():
                sim.cores[c].tensor(n)[:] = v
        sim.simulate()
        results = [{"out": np.array(sim.cores[c].tensor("out"))} for c in range(N_CORES)]
    else:
        trace = bool(int(os.environ.get("KERNEL_TRACE", "0")))
        res = run_bass_kernel_spmd(nc, in_maps, core_ids=list(range(N_CORES)), trace=trace)
        if trace:
            kernel.last_exec_time_ns = res.exec_time_ns
            kernel.last_results = res
        results = res.results

    out = np.concatenate([results[c]["out"][0] for c in range(N_CORES)])[:NP_]
    return out.reshape(NP_, 1).astype(np.float32)


# revision 40
# speedup vs baseline: 1.4805x; 1.0124x over previous
import sys
import os
sys.path.insert(0, '/opt/trn_rl_repo')
import numpy as np
import ml_dtypes

BF = ml_dtypes.bfloat16

N_CORES = 8
GB = int(os.environ.get("KERNEL_GB", "8"))    # tiles per gather batch
MSGB = int(os.environ.get("KERNEL_MSGB", "16"))  # msg pool buffers
WBLK = 64          # one-hot window width
SWBLK = 512        # psum superwindow (one PSUM bank of fp32)
CBLK = 32          # tiles per DVE is_equal batch
DLG = 128          # tiles per dstloc DMA (multiple of CBLK)
IDXG = 4           # gather batches per idx DMA slab
SCRATCH = int(os.environ.get("KERNEL_SCRATCH", "98304"))    # SWDGE ring bytes -> 4096 descriptors (2 batches in flight)

NA, NP_, FIN, H = 100000, 150000, 128, 64
SA, SP = NA // N_CORES, NP_ // N_CORES          # 12500, 18750
SPLIT = 6144                                    # catA rows/core (12 PA blocks)
SPLIT_B = SA - SPLIT                            # 6356
CATA_N = SPLIT * N_CORES                        # 49152
CATB_N = SPLIT_B * N_CORES                      # 50848
BOUND_PA = np.arange(0, NP_ + 24999, 25000)[:7]             # 6 chunks of 25000
BOUND_S0 = np.array([0, CATA_N // 2, CATA_N])               # 2 x 24576
BOUND_S1 = np.array([0, CATB_N // 2, CATB_N])               # 2 x 25424


def _ceil(a, b):
    return -(-a // b)


def _wrap_idx_batch(arr):
    """[GB*128] int16 -> [128, GB*8] staged layout: idx i at (i%16, i//16), x8."""
    n = arr.shape[0]
    w16 = arr.reshape(n // 16, 16).T
    return np.tile(w16, (8, 1))


def _build_pass(dst, src_gid, sel, S, bounds):
    """One gather/aggregate pass: edges [sel], dst sharded by S across cores,
    src gathered from a table addressed by src_gid with chunk boundaries."""
    idx_e = np.nonzero(sel)[0]
    d = dst[idx_e].astype(np.int64)
    s = src_gid[idx_e].astype(np.int64)
    core = d // S
    dst_rel = d - core * S
    nch = len(bounds) - 1
    k_arr = np.searchsorted(bounds, s, side='right') - 1
    src_loc = (s - bounds[k_arr]).astype(np.int16)
    nblk64 = _ceil(S, WBLK)
    nblk512 = _ceil(S, SWBLK)
    m_arr = dst_rel // WBLK
    bucket = m_arr * nch + k_arr
    nbuckets = nblk64 * nch

    flat = core * nbuckets + bucket
    counts = np.bincount(flat, minlength=N_CORES * nbuckets).reshape(N_CORES, nbuckets)
    slots = _ceil(counts, 128).max(axis=0)
    ntiles = int(slots.sum())
    bucket_tile_start = np.zeros(nbuckets + 1, np.int64)
    np.cumsum(slots, out=bucket_tile_start[1:])

    tile_bucket = np.repeat(np.arange(nbuckets), slots)
    tile_m = tile_bucket // nch
    tile_chunk = (tile_bucket % nch).astype(np.int64)
    tile_j = tile_m // (SWBLK // WBLK)
    tile_off = (tile_m % (SWBLK // WBLK)) * WBLK

    blk_start = np.searchsorted(tile_j, np.arange(nblk512), side='left')
    blk_end = np.searchsorted(tile_j, np.arange(nblk512), side='right')

    # per-window psum start/stop flags (window = (j, m) run of tiles)
    w_start = np.ones(ntiles, bool)
    w_stop = np.ones(ntiles, bool)
    if ntiles > 1:
        same = tile_m[1:] == tile_m[:-1]
        w_start[1:] = ~same
        w_stop[:-1] = ~same
    # windows with no tiles at all need explicit psum zeroing
    present = np.zeros(nblk64, bool)
    present[tile_m] = True
    empty_w = [[] for _ in range(nblk512)]
    for m in np.nonzero(~present)[0]:
        empty_w[m // (SWBLK // WBLK)].append(int((m % (SWBLK // WBLK)) * WBLK))

    # max fill (over cores) of each tile; only a bucket's last tile is partial
    tile_fill = np.full(ntiles, 128, np.int64)
    cmax = counts.max(axis=0)
    for b in np.nonzero(slots)[0]:
        last = bucket_tile_start[b + 1] - 1
        tile_fill[last] = min(max(int(cmax[b]) - 128 * (int(slots[b]) - 1), 1), 128)

    # gather batches per chunk
    chunk_tiles = [np.nonzero(tile_chunk == k)[0] for k in range(nch)]
    nb_per_chunk = [_ceil(len(ct), GB) if len(ct) else 0 for ct in chunk_tiles]
    tile_gb = np.zeros(ntiles, np.int64)
    tile_gslot = np.zeros(ntiles, np.int64)
    batch_sizes = []           # [k][b] -> real tiles in batch
    batch_nidx = []            # [k][b] -> num_idxs (last tile trimmed to 16)
    for k in range(nch):
        ct = chunk_tiles[k]
        # within each batch, emptiest tile last -> largest num_idxs trim
        parts = []
        for b in range(nb_per_chunk[k]):
            seg = ct[b * GB:(b + 1) * GB]
            parts.append(seg[np.argsort(-tile_fill[seg], kind='stable')])
        ct = np.concatenate(parts) if parts else ct
        chunk_tiles[k] = ct
        pos = np.arange(len(ct))
        tile_gb[ct] = pos // GB
        tile_gslot[ct] = pos % GB
        bs = [min(GB, len(ct) - b * GB) for b in range(nb_per_chunk[k])]
        batch_sizes.append(bs)
        bn = []
        for b in range(nb_per_chunk[k]):
            tt = ct[b * GB:(b + 1) * GB]
            bn.append((len(tt) - 1) * 128 + _ceil(int(tile_fill[tt[-1]]), 16) * 16)
        batch_nidx.append(bn)
    batch_base = np.zeros(nch + 1, np.int64)
    np.cumsum(nb_per_chunk, out=batch_base[1:])
    NB = max(int(batch_base[-1]), 1)

    ntiles_pad = _ceil(max(ntiles, 1), DLG) * DLG

    order = np.lexsort((bucket, core))
    flat_sorted = flat[order]
    grp_start = np.searchsorted(flat_sorted, flat_sorted, side='left')
    rank = np.arange(len(idx_e)) - grp_start

    idx_staged, dl_staged = [], []
    core_ord = core[order]
    for c in range(N_CORES):
        selc = order[core_ord == c]
        r_sel = rank[core_ord == c]
        b_sel = bucket[selc]
        gtile = bucket_tile_start[b_sel] + r_sel // 128
        lane = r_sel % 128

        idx_flat = np.zeros(ntiles * 128, np.int16)
        idx_flat[gtile * 128 + lane] = src_loc[selc]
        dl = np.full((128, ntiles_pad), -1.0, BF)
        dl[lane, gtile] = (dst_rel[selc] - tile_m[gtile] * WBLK).astype(BF)

        idx_cols = np.zeros((128, NB * GB * 8), np.int16)
        per_tile = idx_flat.reshape(ntiles, 128)
        for k in range(nch):
            ct = chunk_tiles[k]
            for b in range(nb_per_chunk[k]):
                arr = np.zeros(GB * 128, np.int16)
                tt = ct[b * GB:(b + 1) * GB]
                arr[:len(tt) * 128] = per_tile[tt].reshape(-1)
                c0 = (batch_base[k] + b) * GB * 8
                idx_cols[:, c0:c0 + GB * 8] = _wrap_idx_batch(arr)
        idx_staged.append(idx_cols)
        dl_staged.append(dl)

    meta = dict(S=S, nch=nch, bounds=bounds, nblk512=nblk512, ntiles=ntiles,
                ntiles_pad=ntiles_pad, NB=NB, batch_base=batch_base,
                nb_per_chunk=nb_per_chunk, batch_sizes=batch_sizes,
                batch_nidx=batch_nidx,
                tile_chunk=tile_chunk, tile_off=tile_off, tile_gb=tile_gb,
                tile_gslot=tile_gslot, blk_start=blk_start, blk_end=blk_end,
                w_start=w_start, w_stop=w_stop, empty_w=empty_w)
    return meta, idx_staged, dl_staged


def kernel(author_features, edge_author, edge_paper, paper_emb, Wproj, bproj,
           W1l_ap, b1_ap, W1r_ap, W1l_pa, b1_pa, W1r_pa,
           W2l_ap, b2_ap, W2r_ap, W2l_pa, b2_pa, W2r_pa,
           Wl1, bl1, Wl2, bl2):
    import concourse.bass as bass
    import concourse.tile as tile
    from concourse import bacc, mybir
    from concourse.bass_utils import run_bass_kernel_spmd
    from concourse.library_config import mlp

    f32 = mybir.dt.float32
    bf16 = mybir.dt.bfloat16
    i16 = mybir.dt.int16
    AF = mybir.ActivationFunctionType

    af = np.asarray(author_features, np.float32)
    p = np.asarray(paper_emb, np.float32)
    ea = np.asarray(edge_author, np.int64)
    ep = np.asarray(edge_paper, np.int64)
    E = ea.shape[0]

    # ---- host prep: cat-table row mapping for authors ----
    c_a = ea // SA
    r_a = ea - c_a * SA
    gidA = np.where(r_a < SPLIT, c_a * SPLIT + r_a,
                    CATA_N + c_a * SPLIT_B + (r_a - SPLIT))
    selS0 = gidA < CATA_N

    mPA, idxPA, dlPA = _build_pass(ea, ep, np.ones(E, bool), SA, BOUND_PA)
    mS0, idxS0, dlS0 = _build_pass(ep, gidA, selS0, SP, BOUND_S0)
    mS1, idxS1, dlS1 = _build_pass(ep, gidA - CATA_N, ~selS0, SP, BOUND_S1)

    # ---- staged tables ----
    afT_cores = [np.ascontiguousarray(af[c * SA:(c + 1) * SA].T).astype(BF)
                 for c in range(N_CORES)]
    p_bf = np.zeros((NP_, 128), BF)
    p_bf[:, :64] = p.astype(BF)
    pT_cores = [np.ascontiguousarray(p[c * SP:(c + 1) * SP].T).astype(BF)
                for c in range(N_CORES)]

    invA_cores, invP_cores = [], []
    for c in range(N_CORES):
        cntA = np.bincount(ea[ea // SA == c] - c * SA, minlength=SA).astype(np.float32)
        cntP = np.bincount(ep[ep // SP == c] - c * SP, minlength=SP).astype(np.float32)
        invA_cores.append(np.tile((1.0 / np.maximum(cntA, 1.0))[None, :], (64, 1)).astype(BF))
        invP_cores.append(np.tile((1.0 / np.maximum(cntP, 1.0))[None, :], (128, 1)).astype(BF))

    iota_stage = np.tile(np.arange(WBLK, dtype=np.float32)[None, :], (128, 1)).astype(BF)
    ones_bf = np.ones((1, SA), BF)
    W1rb_pa = np.vstack([np.asarray(W1r_pa, np.float32),
                         np.asarray(b1_pa, np.float32)[None]])

    wb_names = ["Wproj", "W1l_ap", "W1r_ap", "W1l_pa", "W1rb_pa",
                "W2l_ap", "W2r_ap", "Wl1", "Wl2c"]
    wb_vals = [np.asarray(Wproj, np.float32).astype(BF),
               np.asarray(W1l_ap, np.float32).astype(BF),
               np.asarray(W1r_ap, np.float32).astype(BF),
               np.asarray(W1l_pa, np.float32).astype(BF),
               W1rb_pa.astype(BF),
               np.asarray(W2l_ap, np.float32).astype(BF),
               np.asarray(W2r_ap, np.float32).astype(BF),
               np.asarray(Wl1, np.float32).astype(BF),
               np.asarray(Wl2, np.float32).reshape(64, 1).astype(BF)]
    # cat 'a' half is stored WITHOUT bproj; fold W1l_ap^T @ bproj into b1
    b1_eff = (np.asarray(b1_ap, np.float32)
              + np.asarray(W1l_ap, np.float32).T @ np.asarray(bproj, np.float32))
    wf_names = ["bprojc", "b1c", "b2c", "bl1c", "bl2c"]
    wf_vals = [np.asarray(bproj, np.float32).reshape(64, 1),
               b1_eff.reshape(64, 1),
               np.asarray(b2_ap, np.float32).reshape(64, 1),
               np.asarray(bl1, np.float32).reshape(64, 1),
               np.asarray(bl2, np.float32).reshape(1, 1)]

    # ---- build program ----
    nc = bacc.Bacc("TRN2", target_bir_lowering=False, debug=False,
                   num_devices=N_CORES, dynamic_dma_scratch_size=SCRATCH)

    afT_h = nc.dram_tensor("afT", [128, SA], bf16, kind="ExternalInput")
    p_bf_h = nc.dram_tensor("p_bf", [NP_, 128], bf16, kind="ExternalInput")
    pT_h = nc.dram_tensor("pT", [64, SP], bf16, kind="ExternalInput")
    invA_h = nc.dram_tensor("invA", [64, SA], bf16, kind="ExternalInput")
    invP_h = nc.dram_tensor("invP", [128, SP], bf16, kind="ExternalInput")
    idxPA_h = nc.dram_tensor("idxPA", list(idxPA[0].shape), i16, kind="ExternalInput")
    dlPA_h = nc.dram_tensor("dlPA", list(dlPA[0].shape), bf16, kind="ExternalInput")
    idxS0_h = nc.dram_tensor("idxS0", list(idxS0[0].shape), i16, kind="ExternalInput")
    dlS0_h = nc.dram_tensor("dlS0", list(dlS0[0].shape), bf16, kind="ExternalInput")
    idxS1_h = nc.dram_tensor("idxS1", list(idxS1[0].shape), i16, kind="ExternalInput")
    dlS1_h = nc.dram_tensor("dlS1", list(dlS1[0].shape), bf16, kind="ExternalInput")
    ones_h = nc.dram_tensor("ones", [1, SA], bf16, kind="ExternalInput")
    iota_h = nc.dram_tensor("iotaf", [128, WBLK], bf16, kind="ExternalInput")
    wb_handles = {n: nc.dram_tensor(n, list(v.shape), bf16, kind="ExternalInput")
                  for n, v in zip(wb_names, wb_vals)}
    wf_handles = {n: nc.dram_tensor(n, list(v.shape), f32, kind="ExternalInput")
                  for n, v in zip(wf_names, wf_vals)}
    out_h = nc.dram_tensor("out", [1, SP], f32, kind="ExternalOutput")

    aT65_h = nc.dram_tensor("aT65", [65, SA], bf16)
    catA_in = nc.dram_tensor("catA_in", [SPLIT, 128], bf16)
    catB_in = nc.dram_tensor("catB_in", [SPLIT_B, 128], bf16)
    catA_full = nc.dram_tensor("catA_full", [CATA_N, 128], bf16, addr_space="Shared")
    catB_full = nc.dram_tensor("catB_full", [CATB_N, 128], bf16, addr_space="Shared")

    rg = [list(range(N_CORES))]
    nblkA = SA // SWBLK + (1 if SA % SWBLK else 0)      # 25
    nblkP = mS0['nblk512']                              # 37

    with tile.TileContext(nc) as tc:
        import contextlib
        with contextlib.ExitStack() as ctx:
            const = ctx.enter_context(tc.tile_pool(name="const", bufs=1))
            msg_p = ctx.enter_context(tc.tile_pool(name="msg", bufs=MSGB))
            idx_p = ctx.enter_context(tc.tile_pool(name="idx", bufs=14))
            oh_p = ctx.enter_context(tc.tile_pool(name="oh", bufs=3))
            dl_p = ctx.enter_context(tc.tile_pool(name="dl", bufs=3))
            inv_p = ctx.enter_context(tc.tile_pool(name="inv", bufs=3))
            mean_p = ctx.enter_context(tc.tile_pool(name="mean", bufs=3))
            sum_p = ctx.enter_context(tc.tile_pool(name="sum", bufs=2))
            x65_p = ctx.enter_context(tc.tile_pool(name="x65", bufs=3))
            big_p = ctx.enter_context(tc.tile_pool(name="big", bufs=4))
            sml_p = ctx.enter_context(tc.tile_pool(name="sml", bufs=4))
            aft_p = ctx.enter_context(tc.tile_pool(name="aft", bufs=2))
            outr_p = ctx.enter_context(tc.tile_pool(name="outr", bufs=2))
            ps0_p = ctx.enter_context(tc.tile_pool(name="ps0", bufs=int(os.environ.get("KERNEL_PS0B", "3")), space="PSUM"))
            psE_p = ctx.enter_context(tc.tile_pool(name="psE", bufs=2, space="PSUM"))
            psA_p = ctx.enter_context(tc.tile_pool(name="psA", bufs=2, space="PSUM"))
            psO_p = ctx.enter_context(tc.tile_pool(name="psO", bufs=1, space="PSUM"))

            nc.gpsimd.load_library(mlp)
            gcount = [0]
            _regs = {}

            def nreg(v):
                if v not in _regs:
                    _regs[v] = nc.gpsimd.to_reg(v)
                return _regs[v]

            nreg(GB * 128)

            wt = {}
            for n, v in zip(wb_names, wb_vals):
                t = const.tile(list(v.shape), bf16, tag=f"w_{n}")
                nc.sync.dma_start(t[:], wb_handles[n][:])
                wt[n] = t
            for n, v in zip(wf_names, wf_vals):
                t = const.tile(list(v.shape), f32, tag=f"w_{n}")
                nc.sync.dma_start(t[:], wf_handles[n][:])
                wt[n] = t

            iota_f = const.tile([128, WBLK], bf16, tag="iota_f")
            nc.sync.dma_start(iota_f[:], iota_h[:])
            zc1 = const.tile([1, 128], bf16, tag="zc1")
            nc.vector.memset(zc1[:], 0.0)
            zr64 = const.tile([1, WBLK], bf16, tag="zr64")
            nc.vector.memset(zr64[:], 0.0)
            nc.sync.dma_start(aT65_h[64:65, :], ones_h[:])
            partial = const.tile([128, nblkP * SWBLK], bf16, tag="partial")

            def cat_rows(r0, r1):
                """Map local author rows [r0,r1) to (tensor, row range)."""
                if r1 <= SPLIT:
                    return catA_in, r0, r1
                assert r0 >= SPLIT
                return catB_in, r0 - SPLIT, r1 - SPLIT

            # ---- projection block: a = af @ Wproj (+bproj only in aT65) ----
            def proj_block(b):
                c0 = b * SWBLK
                cw = min(SWBLK, SA - c0)
                afT_t = aft_p.tile([128, SWBLK], bf16, tag="aft", name=f"afT_{b}")
                nc.sync.dma_start(afT_t[:, :cw], afT_h[:, c0:c0 + cw])
                ps = psE_p.tile([64, SWBLK], f32, tag="psE")
                nc.tensor.matmul(ps[:, :cw], wt["Wproj"][:], afT_t[:, :cw],
                                 start=True, stop=True)
                aTw = big_p.tile([64, SWBLK], bf16, tag="big")
                nc.scalar.activation(aTw[:, :cw], ps[:, :cw], AF.Identity,
                                     bias=wt["bprojc"][:])
                nc.sync.dma_start(aT65_h[0:64, c0:c0 + cw], aTw[:, :cw])
                for s in range(0, cw, 128):
                    ws = min(128, cw - s)
                    psr = psA_p.tile([128, 64], f32, tag="psA")
                    nc.tensor.matmul(psr[:ws, :], afT_t[:, s:s + ws], wt["Wproj"][:],
                                     start=True, stop=True)
                    art = sml_p.tile([128, 64], bf16, tag="sml")
                    nc.scalar.activation(art[:ws, :], psr[:ws, :], AF.Identity)
                    tgt, t0r, t1r = cat_rows(c0 + s, c0 + s + ws)
                    nc.sync.dma_start(tgt[t0r:t1r, 0:64], art[:ws, :])

            proj_block(0)
            proj_block(1)

            # ---- generic gather/aggregate pass ----
            def emit_pass(meta, idx_h, dl_h, src_tab_fn, close_block, pname,
                          extra=None):
                nch = meta['nch']
                gb_cur = [-1] * nch
                ig_t = [dict() for _ in range(nch)]
                msg_t = [None] * nch
                cb_cur, oh_t = [-1], [None]
                dl_cur, dl_t = [-1], [None]

                def load_slab(k, ig):
                    t = idx_p.tile([128, IDXG * GB * 8], i16, tag="idx",
                                   name=f"idx{pname}_{k}_{ig}")
                    icol = (int(meta['batch_base'][k]) + ig * IDXG) * GB * 8
                    nb_k = int(meta['nb_per_chunk'][k])
                    nld = min(IDXG, nb_k - ig * IDXG) * GB * 8
                    nc.sync.dma_start(t[:, :nld], idx_h[:, icol:icol + nld])
                    ig_t[k][ig] = t
                for j in range(meta['nblk512']):
                    c0 = j * SWBLK
                    C = min(SWBLK, meta['S'] - c0)
                    ps0 = ps0_p.tile([128, SWBLK], f32, tag="ps0")
                    for off in meta['empty_w'][j]:
                        nc.tensor.matmul(ps0[:, off:off + WBLK], zc1[:], zr64[:],
                                         start=True, stop=True)
                    t0, t1 = int(meta['blk_start'][j]), int(meta['blk_end'][j])
                    for t in range(t0, t1):
                        k = int(meta['tile_chunk'][t])
                        gb = int(meta['tile_gb'][t])
                        if gb != gb_cur[k]:
                            gb_cur[k] = gb
                            ig = gb // IDXG
                            nb_k = int(meta['nb_per_chunk'][k])
                            if ig not in ig_t[k]:
                                load_slab(k, ig)
                            if gb % IDXG == 0 and (ig + 1) * IDXG < nb_k \
                                    and (ig + 1) not in ig_t[k]:
                                load_slab(k, ig + 1)
                            if ig - 1 in ig_t[k]:
                                del ig_t[k][ig - 1]
                            msg_t[k] = msg_p.tile([128, GB, 128], bf16, tag="msg",
                                                  name=f"msg{pname}_{k}_{gb}")
                            goff = (gb % IDXG) * GB * 8
                            tab, kbase, kend = src_tab_fn(k)
                            nreal = meta['batch_sizes'][k][gb]
                            gcount[0] += 1
                            if gcount[0] <= MSGB:
                                # first pass through the msg pool: gather full
                                # batches so every buffer lane is initialized
                                # (later trims may leave stale lanes; they are
                                # zeroed by the one-hot, so must be finite)
                                nidx = GB * 128
                                nreal = GB
                            else:
                                nidx = int(meta['batch_nidx'][k][gb])
                            _gi = nc.gpsimd.dma_gather(msg_t[k][:, :nreal, :],
                                                       tab[kbase:kend, :],
                                                       ig_t[k][ig][:, goff:goff + GB * 8],
                                                       nidx, nreg(nidx), 128)
                            if os.environ.get("KERNEL_FOLLOW") and pname == "pa":
                                if 70 <= gcount[0] <= 78:
                                    tile.tile_follow(_gi, log_all_deps=True)
                        cb = t // CBLK
                        if cb != cb_cur[0]:
                            cb_cur[0] = cb
                            dg = t // DLG
                            if dg != dl_cur[0]:
                                dl_cur[0] = dg
                                dl_t[0] = dl_p.tile([128, DLG], bf16, tag="dl",
                                                    name=f"dl{pname}_{dg}")
                                nc.scalar.dma_start(dl_t[0][:],
                                                    dl_h[:, dg * DLG:(dg + 1) * DLG])
                            oh_t[0] = oh_p.tile([128, CBLK, WBLK], bf16, tag="oh",
                                                name=f"oh{pname}_{cb}")
                            dcol = (cb * CBLK) % DLG
                            in0 = dl_t[0][:, dcol:dcol + CBLK].to_broadcast(
                                [128, CBLK, WBLK])
                            _i = iota_f[:]
                            in1 = bass.AP(_i.tensor, _i.offset,
                                          [list(_i.ap[0]), [0, CBLK], list(_i.ap[1])])
                            nc.vector.tensor_tensor(out=oh_t[0][:], in0=in0, in1=in1,
                                                    op=mybir.AluOpType.is_equal)
                        off = int(meta['tile_off'][t])
                        nc.tensor.matmul(ps0[:, off:off + WBLK],
                                         msg_t[k][:, int(meta['tile_gslot'][t]), :],
                                         oh_t[0][:, t % CBLK, :],
                                         start=bool(meta['w_start'][t]),
                                         stop=bool(meta['w_stop'][t]))
                    close_block(j, c0, C, ps0)
                    if extra is not None:
                        extra(j)

            # ---- PA pass: papers -> authors, produces a1 into cat tables ----
            def close_pa(j, c0, C, ps0):
                inv_t = inv_p.tile([64, SWBLK], bf16, tag="inv64")
                nc.scalar.dma_start(inv_t[:, :C], invA_h[:, c0:c0 + C])
                meanT = mean_p.tile([64, SWBLK], bf16, tag="mean64")
                nc.vector.tensor_tensor(out=meanT[:, :C], in0=ps0[0:64, :C],
                                        in1=inv_t[:, :C], op=mybir.AluOpType.mult)
                aT_t = x65_p.tile([65, SWBLK], bf16, tag="x65")
                nc.scalar.dma_start(aT_t[:, :C], aT65_h[:, c0:c0 + C])
                for s in range(0, C, 128):
                    ws = min(128, C - s)
                    psA = psA_p.tile([128, 64], f32, tag="psA")
                    nc.tensor.matmul(psA[:ws, :], meanT[:, s:s + ws], wt["W1l_pa"][:],
                                     start=True, stop=False)
                    nc.tensor.matmul(psA[:ws, :], aT_t[:, s:s + ws], wt["W1rb_pa"][:],
                                     start=False, stop=True)
                    o = sml_p.tile([128, 64], bf16, tag="sml")
                    nc.scalar.activation(o[:ws, :], psA[:ws, :], AF.Relu)
                    tgt, t0r, t1r = cat_rows(c0 + s, c0 + s + ws)
                    nc.sync.dma_start(tgt[t0r:t1r, 64:128], o[:ws, :])
                if STAGE < 3:
                    return
                if j == SPLIT // SWBLK - 1:
                    nc.gpsimd.collective_compute("AllGather", mybir.AluOpType.bypass,
                                                 replica_groups=rg, ins=[catA_in[:]],
                                                 outs=[catA_full[:]])
                if j == nblkA - 1:
                    nc.gpsimd.collective_compute("AllGather", mybir.AluOpType.bypass,
                                                 replica_groups=rg, ins=[catB_in[:]],
                                                 outs=[catB_full[:]])

            STAGE = int(os.environ.get("KERNEL_STAGE", "5"))

            def pa_extra(j):
                if j + 2 < nblkA:
                    proj_block(j + 2)

            if STAGE >= 2:
                emit_pass(mPA, idxPA_h, dlPA_h,
                          lambda k: (p_bf_h, k * 25000, min((k + 1) * 25000, NP_)),
                          close_pa, "pa", extra=pa_extra)

            # ---- AP sweep 0: accumulate catA contributions into partials ----
            def close_s0(j, c0, C, ps0):
                nc.scalar.activation(partial[:, c0:c0 + C], ps0[:, :C], AF.Identity)

            if STAGE >= 4:
                emit_pass(mS0, idxS0_h, dlS0_h,
                          lambda k: (catA_full, int(BOUND_S0[k]), int(BOUND_S0[k + 1])),
                          close_s0, "s0")

            # ---- AP sweep 1 + fused conv1/conv2/head epilogue ----
            def close_s1(j, c0, C, ps0):
                sum_t = sum_p.tile([128, SWBLK], f32, tag="sum")
                nc.vector.tensor_tensor(out=sum_t[:, :C], in0=ps0[:, :C],
                                        in1=partial[:, c0:c0 + C],
                                        op=mybir.AluOpType.add)
                inv_t = inv_p.tile([128, SWBLK], bf16, tag="inv128")
                nc.scalar.dma_start(inv_t[:, :C], invP_h[:, c0:c0 + C])
                meanA = mean_p.tile([64, SWBLK], bf16, tag="meanA")
                nc.vector.tensor_tensor(out=meanA[:, :C], in0=sum_t[0:64, :C],
                                        in1=inv_t[0:64, :C], op=mybir.AluOpType.mult)
                meanB = mean_p.tile([64, SWBLK], bf16, tag="meanB")
                nc.vector.tensor_tensor(out=meanB[:, :C], in0=sum_t[64:128, :C],
                                        in1=inv_t[64:128, :C], op=mybir.AluOpType.mult)
                pT_t = x65_p.tile([64, SWBLK], bf16, tag="pTt")
                nc.sync.dma_start(pT_t[:, :C], pT_h[:, c0:c0 + C])
                psE = psE_p.tile([64, SWBLK], f32, tag="psE")
                nc.tensor.matmul(psE[:, :C], wt["W1l_ap"][:], meanA[:, :C],
                                 start=True, stop=False)
                nc.tensor.matmul(psE[:, :C], wt["W1r_ap"][:], pT_t[:, :C],
                                 start=False, stop=True)
                p1 = big_p.tile([64, SWBLK], bf16, tag="big")
                nc.scalar.activation(p1[:, :C], psE[:, :C], AF.Relu, bias=wt["b1c"][:])
                psE2 = psE_p.tile([64, SWBLK], f32, tag="psE")
                nc.tensor.matmul(psE2[:, :C], wt["W2l_ap"][:], meanB[:, :C],
                                 start=True, stop=False)
                nc.tensor.matmul(psE2[:, :C], wt["W2r_ap"][:], p1[:, :C],
                                 start=False, stop=True)
                p2 = big_p.tile([64, SWBLK], bf16, tag="big")
                nc.scalar.activation(p2[:, :C], psE2[:, :C], AF.Relu, bias=wt["b2c"][:])
                psH = psE_p.tile([64, SWBLK], f32, tag="psE")
                nc.tensor.matmul(psH[:, :C], wt["Wl1"][:], p2[:, :C],
                                 start=True, stop=True)
                h = big_p.tile([64, SWBLK], bf16, tag="big")
                nc.scalar.activation(h[:, :C], psH[:, :C], AF.Relu, bias=wt["bl1c"][:])
                psO = psO_p.tile([1, SWBLK], f32, tag="psO")
                nc.tensor.matmul(psO[:, :C], wt["Wl2c"][:], h[:, :C],
                                 start=True, stop=True)
                o = outr_p.tile([1, SWBLK], f32, tag="outrow")
                nc.scalar.activation(o[:, :C], psO[:, :C], AF.Identity,
                                     bias=wt["bl2c"][:])
                nc.sync.dma_start(out_h[:, c0:c0 + C], o[:, :C])

            if STAGE >= 5:
                emit_pass(mS1, idxS1_h, dlS1_h,
                          lambda k: (catB_full, int(BOUND_S1[k]), int(BOUND_S1[k + 1])),
                          close_s1, "s1")

    nc.compile()

    if int(os.environ.get("KERNEL_TLSIM", "1")):
        try:
            from concourse.timeline_sim import TimelineSim
            _t = TimelineSim(nc)
            kernel.modeled_time_ns = _t.simulate()
            print(f"[kernel] TimelineSim modeled core time: "
                  f"{kernel.modeled_time_ns / 1e3:.1f} us")
        except Exception as e:
            print(f"[kernel] TimelineSim failed: {e}")
            kernel.modeled_time_ns = None

    globals()["_last_nc"] = nc
    if int(os.environ.get("KERNEL_BUILD_ONLY", "0")):
        raise SystemExit(0)

    in_maps = []
    for c in range(N_CORES):
        m = {"afT": afT_cores[c], "p_bf": p_bf, "pT": pT_cores[c],
             "invA": invA_cores[c], "invP": invP_cores[c],
             "idxPA": idxPA[c], "dlPA": dlPA[c],
             "idxS0": idxS0[c], "dlS0": dlS0[c],
             "idxS1": idxS1[c], "dlS1": dlS1[c],
             "ones": ones_bf, "iotaf": iota_stage}
        for n, v in zip(wb_names, wb_vals):
            m[n] = v
        for n, v in zip(wf_names, wf_vals):
            m[n] = v
        in_maps.append(m)

    if int(os.environ.get("KERNEL_SIM", "0")):
        from concourse import bass_interp
        sim = bass_interp.MultiCoreSim(nc, N_CORES)
        for c in range(N_CORES):
            for n, v in in_maps[c].items():
                sim.cores[c].tensor(n)[:] = v
        sim.simulate()
        results = [{"out": np.array(sim.cores[c].tensor("out"))} for c in range(N_CORES)]
    else:
        trace = bool(int(os.environ.get("KERNEL_TRACE", "0")))
        res = run_bass_kernel_spmd(nc, in_maps, core_ids=list(range(N_CORES)), trace=trace)
        if trace:
            kernel.last_exec_time_ns = res.exec_time_ns
            kernel.last_results = res
        results = res.results

    out = np.concatenate([results[c]["out"][0] for c in range(N_CORES)])[:NP_]
    return out.reshape(NP_, 1).astype(np.float32)


# revision 66
# speedup vs baseline: 1.5145x; 1.0230x over previous
import sys
import os
sys.path.insert(0, '/opt/trn_rl_repo')
import numpy as np
import ml_dtypes

BF = ml_dtypes.bfloat16

N_CORES = 8
GB = int(os.environ.get("KERNEL_GB", "8"))    # tiles per gather batch
MSGB = int(os.environ.get("KERNEL_MSGB", "16"))  # msg pool buffers
NSWQ = int(os.environ.get("KERNEL_NSWQ", "1"))   # SWDGE queues
WBLK = 64          # one-hot window width
SWBLK = 512        # psum superwindow (one PSUM bank of fp32)
CBLK = 32          # tiles per DVE is_equal batch
DLG = 128          # tiles per dstloc DMA (multiple of CBLK)
IDXG = 4           # gather batches per idx DMA slab
SCRATCH = int(os.environ.get("KERNEL_SCRATCH", "98304"))    # SWDGE ring bytes -> 4096 descriptors (2 batches in flight)

NA, NP_, FIN, H = 100000, 150000, 128, 64
SA, SP = NA // N_CORES, NP_ // N_CORES          # 12500, 18750
SPLIT = 6144                                    # catA rows/core (12 PA blocks)
SPLIT_B = SA - SPLIT                            # 6356
CATA_N = SPLIT * N_CORES                        # 49152
CATB_N = SPLIT_B * N_CORES                      # 50848
BOUND_PA = np.arange(0, NP_ + 24999, 25000)[:7]             # 6 chunks of 25000
BOUND_S0 = np.array([0, CATA_N // 2, CATA_N])               # 2 x 24576
BOUND_S1 = np.array([0, CATB_N // 2, CATB_N])               # 2 x 25424


def _ceil(a, b):
    return -(-a // b)


def _wrap_idx_batch(arr):
    """[GB*128] int16 -> [128, GB*8] staged layout: idx i at (i%16, i//16), x8."""
    n = arr.shape[0]
    w16 = arr.reshape(n // 16, 16).T
    return np.tile(w16, (8, 1))


def _build_pass(dst, src_gid, sel, S, bounds):
    """One gather/aggregate pass: edges [sel], dst sharded by S across cores,
    src gathered from a table addressed by src_gid with chunk boundaries."""
    idx_e = np.nonzero(sel)[0]
    d = dst[idx_e].astype(np.int64)
    s = src_gid[idx_e].astype(np.int64)
    core = d // S
    dst_rel = d - core * S
    nch = len(bounds) - 1
    k_arr = np.searchsorted(bounds, s, side='right') - 1
    src_loc = (s - bounds[k_arr]).astype(np.int16)
    nblk64 = _ceil(S, WBLK)
    nblk512 = _ceil(S, SWBLK)
    m_arr = dst_rel // WBLK
    bucket = m_arr * nch + k_arr
    nbuckets = nblk64 * nch

    flat = core * nbuckets + bucket
    counts = np.bincount(flat, minlength=N_CORES * nbuckets).reshape(N_CORES, nbuckets)
    slots = _ceil(counts, 128).max(axis=0)
    ntiles = int(slots.sum())
    bucket_tile_start = np.zeros(nbuckets + 1, np.int64)
    np.cumsum(slots, out=bucket_tile_start[1:])

    tile_bucket = np.repeat(np.arange(nbuckets), slots)
    tile_m = tile_bucket // nch
    tile_chunk = (tile_bucket % nch).astype(np.int64)
    tile_j = tile_m // (SWBLK // WBLK)
    tile_off = (tile_m % (SWBLK // WBLK)) * WBLK

    blk_start = np.searchsorted(tile_j, np.arange(nblk512), side='left')
    blk_end = np.searchsorted(tile_j, np.arange(nblk512), side='right')

    # per-window psum start/stop flags (window = (j, m) run of tiles)
    w_start = np.ones(ntiles, bool)
    w_stop = np.ones(ntiles, bool)
    if ntiles > 1:
        same = tile_m[1:] == tile_m[:-1]
        w_start[1:] = ~same
        w_stop[:-1] = ~same
    # windows with no tiles at all need explicit psum zeroing
    present = np.zeros(nblk64, bool)
    present[tile_m] = True
    empty_w = [[] for _ in range(nblk512)]
    for m in np.nonzero(~present)[0]:
        empty_w[m // (SWBLK // WBLK)].append(int((m % (SWBLK // WBLK)) * WBLK))

    # max fill (over cores) of each tile; only a bucket's last tile is partial
    tile_fill = np.full(ntiles, 128, np.int64)
    cmax = counts.max(axis=0)
    for b in np.nonzero(slots)[0]:
        last = bucket_tile_start[b + 1] - 1
        tile_fill[last] = min(max(int(cmax[b]) - 128 * (int(slots[b]) - 1), 1), 128)

    # gather batches per chunk
    chunk_tiles = [np.nonzero(tile_chunk == k)[0] for k in range(nch)]
    nb_per_chunk = [_ceil(len(ct), GB) if len(ct) else 0 for ct in chunk_tiles]
    tile_gb = np.zeros(ntiles, np.int64)
    tile_gslot = np.zeros(ntiles, np.int64)
    batch_sizes = []           # [k][b] -> real tiles in batch
    batch_nidx = []            # [k][b] -> num_idxs (last tile trimmed to 16)
    for k in range(nch):
        ct = chunk_tiles[k]
        # within each batch, emptiest tile last -> largest num_idxs trim
        parts = []
        for b in range(nb_per_chunk[k]):
            seg = ct[b * GB:(b + 1) * GB]
            parts.append(seg[np.argsort(-tile_fill[seg], kind='stable')])
        ct = np.concatenate(parts) if parts else ct
        chunk_tiles[k] = ct
        pos = np.arange(len(ct))
        tile_gb[ct] = pos // GB
        tile_gslot[ct] = pos % GB
        bs = [min(GB, len(ct) - b * GB) for b in range(nb_per_chunk[k])]
        batch_sizes.append(bs)
        bn = []
        for b in range(nb_per_chunk[k]):
            tt = ct[b * GB:(b + 1) * GB]
            bn.append((len(tt) - 1) * 128 + _ceil(int(tile_fill[tt[-1]]), 16) * 16)
        batch_nidx.append(bn)
    batch_base = np.zeros(nch + 1, np.int64)
    np.cumsum(nb_per_chunk, out=batch_base[1:])
    NB = max(int(batch_base[-1]), 1)

    ntiles_pad = _ceil(max(ntiles, 1), DLG) * DLG

    order = np.lexsort((bucket, core))
    flat_sorted = flat[order]
    grp_start = np.searchsorted(flat_sorted, flat_sorted, side='left')
    rank = np.arange(len(idx_e)) - grp_start

    idx_staged, dl_staged = [], []
    core_ord = core[order]
    for c in range(N_CORES):
        selc = order[core_ord == c]
        r_sel = rank[core_ord == c]
        b_sel = bucket[selc]
        gtile = bucket_tile_start[b_sel] + r_sel // 128
        lane = r_sel % 128

        idx_flat = np.zeros(ntiles * 128, np.int16)
        idx_flat[gtile * 128 + lane] = src_loc[selc]
        dl = np.full((128, ntiles_pad), -1.0, BF)
        dl[lane, gtile] = (dst_rel[selc] - tile_m[gtile] * WBLK).astype(BF)

        idx_cols = np.zeros((128, NB * GB * 8), np.int16)
        per_tile = idx_flat.reshape(ntiles, 128)
        for k in range(nch):
            ct = chunk_tiles[k]
            for b in range(nb_per_chunk[k]):
                arr = np.zeros(GB * 128, np.int16)
                tt = ct[b * GB:(b + 1) * GB]
                arr[:len(tt) * 128] = per_tile[tt].reshape(-1)
                c0 = (batch_base[k] + b) * GB * 8
                idx_cols[:, c0:c0 + GB * 8] = _wrap_idx_batch(arr)
        idx_staged.append(idx_cols)
        dl_staged.append(dl)

    meta = dict(S=S, nch=nch, bounds=bounds, nblk512=nblk512, ntiles=ntiles,
                ntiles_pad=ntiles_pad, NB=NB, batch_base=batch_base,
                nb_per_chunk=nb_per_chunk, batch_sizes=batch_sizes,
                batch_nidx=batch_nidx,
                tile_chunk=tile_chunk, tile_off=tile_off, tile_gb=tile_gb,
                tile_gslot=tile_gslot, blk_start=blk_start, blk_end=blk_end,
                w_start=w_start, w_stop=w_stop, empty_w=empty_w)
    return meta, idx_staged, dl_staged


def kernel(author_features, edge_author, edge_paper, paper_emb, Wproj, bproj,
           W1l_ap, b1_ap, W1r_ap, W1l_pa, b1_pa, W1r_pa,
           W2l_ap, b2_ap, W2r_ap, W2l_pa, b2_pa, W2r_pa,
           Wl1, bl1, Wl2, bl2):
    import concourse.bass as bass
    import concourse.tile as tile
    from concourse import bacc, mybir
    from concourse.bass_utils import run_bass_kernel_spmd
    from concourse.library_config import mlp

    f32 = mybir.dt.float32
    bf16 = mybir.dt.bfloat16
    i16 = mybir.dt.int16
    AF = mybir.ActivationFunctionType

    af = np.asarray(author_features, np.float32)
    p = np.asarray(paper_emb, np.float32)
    ea = np.asarray(edge_author, np.int64)
    ep = np.asarray(edge_paper, np.int64)
    E = ea.shape[0]

    # ---- host prep: cat-table row mapping for authors ----
    c_a = ea // SA
    r_a = ea - c_a * SA
    gidA = np.where(r_a < SPLIT, c_a * SPLIT + r_a,
                    CATA_N + c_a * SPLIT_B + (r_a - SPLIT))
    selS0 = gidA < CATA_N

    mPA, idxPA, dlPA = _build_pass(ea, ep, np.ones(E, bool), SA, BOUND_PA)
    mS0, idxS0, dlS0 = _build_pass(ep, gidA, selS0, SP, BOUND_S0)
    mS1, idxS1, dlS1 = _build_pass(ep, gidA - CATA_N, ~selS0, SP, BOUND_S1)

    # ---- staged tables ----
    afT_cores = [np.ascontiguousarray(af[c * SA:(c + 1) * SA].T).astype(BF)
                 for c in range(N_CORES)]
    p_bf = np.zeros((NP_, 128), BF)
    p_bf[:, :64] = p.astype(BF)
    pT_cores = [np.ascontiguousarray(p[c * SP:(c + 1) * SP].T).astype(BF)
                for c in range(N_CORES)]

    invA_cores, invP_cores = [], []
    for c in range(N_CORES):
        cntA = np.bincount(ea[ea // SA == c] - c * SA, minlength=SA).astype(np.float32)
        cntP = np.bincount(ep[ep // SP == c] - c * SP, minlength=SP).astype(np.float32)
        invA_cores.append(np.tile((1.0 / np.maximum(cntA, 1.0))[None, :], (64, 1)).astype(BF))
        invP_cores.append(np.tile((1.0 / np.maximum(cntP, 1.0))[None, :], (128, 1)).astype(BF))

    iota_stage = np.tile(np.arange(WBLK, dtype=np.float32)[None, :], (128, 1)).astype(BF)
    ones_bf = np.ones((1, SA), BF)
    W1rb_pa = np.vstack([np.asarray(W1r_pa, np.float32),
                         np.asarray(b1_pa, np.float32)[None]])

    wb_names = ["Wproj", "W1l_ap", "W1r_ap", "W1l_pa", "W1rb_pa",
                "W2l_ap", "W2r_ap", "Wl1", "Wl2c"]
    wb_vals = [np.asarray(Wproj, np.float32).astype(BF),
               np.asarray(W1l_ap, np.float32).astype(BF),
               np.asarray(W1r_ap, np.float32).astype(BF),
               np.asarray(W1l_pa, np.float32).astype(BF),
               W1rb_pa.astype(BF),
               np.asarray(W2l_ap, np.float32).astype(BF),
               np.asarray(W2r_ap, np.float32).astype(BF),
               np.asarray(Wl1, np.float32).astype(BF),
               np.asarray(Wl2, np.float32).reshape(64, 1).astype(BF)]
    # cat 'a' half is stored WITHOUT bproj; fold W1l_ap^T @ bproj into b1
    b1_eff = (np.asarray(b1_ap, np.float32)
              + np.asarray(W1l_ap, np.float32).T @ np.asarray(bproj, np.float32))
    wf_names = ["bprojc", "b1c", "b2c", "bl1c", "bl2c"]
    wf_vals = [np.asarray(bproj, np.float32).reshape(64, 1),
               b1_eff.reshape(64, 1),
               np.asarray(b2_ap, np.float32).reshape(64, 1),
               np.asarray(bl1, np.float32).reshape(64, 1),
               np.asarray(bl2, np.float32).reshape(1, 1)]

    # ---- build program ----
    nc = bacc.Bacc("TRN2", target_bir_lowering=False, debug=False,
                   num_devices=N_CORES, dynamic_dma_scratch_size=SCRATCH,
                   num_swdge_queues=NSWQ)

    afT_h = nc.dram_tensor("afT", [128, SA], bf16, kind="ExternalInput")
    p_bf_h = nc.dram_tensor("p_bf", [NP_, 128], bf16, kind="ExternalInput")
    pT_h = nc.dram_tensor("pT", [64, SP], bf16, kind="ExternalInput")
    invA_h = nc.dram_tensor("invA", [64, SA], bf16, kind="ExternalInput")
    invP_h = nc.dram_tensor("invP", [128, SP], bf16, kind="ExternalInput")
    idxPA_h = nc.dram_tensor("idxPA", list(idxPA[0].shape), i16, kind="ExternalInput")
    dlPA_h = nc.dram_tensor("dlPA", list(dlPA[0].shape), bf16, kind="ExternalInput")
    idxS0_h = nc.dram_tensor("idxS0", list(idxS0[0].shape), i16, kind="ExternalInput")
    dlS0_h = nc.dram_tensor("dlS0", list(dlS0[0].shape), bf16, kind="ExternalInput")
    idxS1_h = nc.dram_tensor("idxS1", list(idxS1[0].shape), i16, kind="ExternalInput")
    dlS1_h = nc.dram_tensor("dlS1", list(dlS1[0].shape), bf16, kind="ExternalInput")
    ones_h = nc.dram_tensor("ones", [1, SA], bf16, kind="ExternalInput")
    iota_h = nc.dram_tensor("iotaf", [128, WBLK], bf16, kind="ExternalInput")
    wb_handles = {n: nc.dram_tensor(n, list(v.shape), bf16, kind="ExternalInput")
                  for n, v in zip(wb_names, wb_vals)}
    wf_handles = {n: nc.dram_tensor(n, list(v.shape), f32, kind="ExternalInput")
                  for n, v in zip(wf_names, wf_vals)}
    out_h = nc.dram_tensor("out", [1, SP], f32, kind="ExternalOutput")

    aT65_h = nc.dram_tensor("aT65", [65, SA], bf16)
    catA_in = nc.dram_tensor("catA_in", [SPLIT, 128], bf16)
    catB_in = nc.dram_tensor("catB_in", [SPLIT_B, 128], bf16)
    catA_in_a = nc.dram_tensor("catA_in_a", [SPLIT, 64], bf16)
    catA_in_a1 = nc.dram_tensor("catA_in_a1", [SPLIT, 64], bf16)
    catB_in_a = nc.dram_tensor("catB_in_a", [SPLIT_B, 64], bf16)
    catB_in_a1 = nc.dram_tensor("catB_in_a1", [SPLIT_B, 64], bf16)
    catA_full = nc.dram_tensor("catA_full", [CATA_N, 128], bf16, addr_space="Shared")
    catB_full = nc.dram_tensor("catB_full", [CATB_N, 128], bf16, addr_space="Shared")

    rg = [list(range(N_CORES))]
    nblkA = SA // SWBLK + (1 if SA % SWBLK else 0)      # 25
    nblkP = mS0['nblk512']                              # 37

    with tile.TileContext(nc) as tc:
        import contextlib
        with contextlib.ExitStack() as ctx:
            const = ctx.enter_context(tc.tile_pool(name="const", bufs=1))
            msg_p = ctx.enter_context(tc.tile_pool(name="msg", bufs=MSGB))
            idx_p = ctx.enter_context(tc.tile_pool(name="idx", bufs=14))
            oh_p = ctx.enter_context(tc.tile_pool(name="oh", bufs=3))
            dl_p = ctx.enter_context(tc.tile_pool(name="dl", bufs=3))
            inv_p = ctx.enter_context(tc.tile_pool(name="inv", bufs=3))
            mean_p = ctx.enter_context(tc.tile_pool(name="mean", bufs=3))
            sum_p = ctx.enter_context(tc.tile_pool(name="sum", bufs=2))
            x65_p = ctx.enter_context(tc.tile_pool(name="x65", bufs=3))
            big_p = ctx.enter_context(tc.tile_pool(name="big", bufs=4))
            sml_p = ctx.enter_context(tc.tile_pool(name="sml", bufs=4))
            aft_p = ctx.enter_context(tc.tile_pool(name="aft", bufs=2))
            outr_p = ctx.enter_context(tc.tile_pool(name="outr", bufs=2))
            ps0_p = ctx.enter_context(tc.tile_pool(name="ps0", bufs=int(os.environ.get("KERNEL_PS0B", "3")), space="PSUM"))
            psE_p = ctx.enter_context(tc.tile_pool(name="psE", bufs=2, space="PSUM"))
            psA_p = ctx.enter_context(tc.tile_pool(name="psA", bufs=2, space="PSUM"))
            psO_p = ctx.enter_context(tc.tile_pool(name="psO", bufs=1, space="PSUM"))

            STAGE = int(os.environ.get("KERNEL_STAGE", "5"))
            CC4 = int(os.environ.get("KERNEL_CC4", "0"))
            nc.gpsimd.load_library(mlp)
            gcount = [0]
            _regs = {}

            def nreg(v):
                if v not in _regs:
                    _regs[v] = nc.gpsimd.to_reg(v)
                return _regs[v]

            nreg(GB * 128)

            wt = {}
            for n, v in zip(wb_names, wb_vals):
                t = const.tile(list(v.shape), bf16, tag=f"w_{n}")
                nc.scalar.dma_start(t[:], wb_handles[n][:])
                wt[n] = t
            for n, v in zip(wf_names, wf_vals):
                t = const.tile(list(v.shape), f32, tag=f"w_{n}")
                nc.scalar.dma_start(t[:], wf_handles[n][:])
                wt[n] = t

            iota_f = const.tile([128, WBLK], bf16, tag="iota_f")
            nc.sync.dma_start(iota_f[:], iota_h[:])
            zc1 = const.tile([1, 128], bf16, tag="zc1")
            nc.vector.memset(zc1[:], 0.0)
            zr64 = const.tile([1, WBLK], bf16, tag="zr64")
            nc.vector.memset(zr64[:], 0.0)
            nc.sync.dma_start(aT65_h[64:65, :], ones_h[:])
            partial = const.tile([128, nblkP * SWBLK], bf16, tag="partial")

            def cat_rows(r0, r1, half):
                """Map local author rows [r0,r1) to (write AP)."""
                if r1 <= SPLIT:
                    if CC4:
                        t = catA_in_a if half == 0 else catA_in_a1
                        return t[r0:r1, :]
                    return catA_in[r0:r1, half * 64:half * 64 + 64]
                assert r0 >= SPLIT
                r0, r1 = r0 - SPLIT, r1 - SPLIT
                if CC4:
                    t = catB_in_a if half == 0 else catB_in_a1
                    return t[r0:r1, :]
                return catB_in[r0:r1, half * 64:half * 64 + 64]

            # ---- projection block: a = af @ Wproj (+bproj only in aT65) ----
            def proj_block(b):
                c0 = b * SWBLK
                cw = min(SWBLK, SA - c0)
                afT_t = aft_p.tile([128, SWBLK], bf16, tag="aft", name=f"afT_{b}")
                nc.sync.dma_start(afT_t[:, :cw], afT_h[:, c0:c0 + cw])
                ps = psE_p.tile([64, SWBLK], f32, tag="psE")
                nc.tensor.matmul(ps[:, :cw], wt["Wproj"][:], afT_t[:, :cw],
                                 start=True, stop=True)
                aTw = big_p.tile([64, SWBLK], bf16, tag="big")
                nc.scalar.activation(aTw[:, :cw], ps[:, :cw], AF.Identity,
                                     bias=wt["bprojc"][:])
                nc.sync.dma_start(aT65_h[0:64, c0:c0 + cw], aTw[:, :cw])
                for s in range(0, cw, 128):
                    ws = min(128, cw - s)
                    psr = psA_p.tile([128, 64], f32, tag="psA")
                    nc.tensor.matmul(psr[:ws, :], afT_t[:, s:s + ws], wt["Wproj"][:],
                                     start=True, stop=True)
                    art = sml_p.tile([128, 64], bf16, tag="sml")
                    nc.scalar.activation(art[:ws, :], psr[:ws, :], AF.Identity)
                    nc.sync.dma_start(cat_rows(c0 + s, c0 + s + ws, 0),
                                      art[:ws, :])

            proj_block(0)
            proj_block(1)

            # ---- generic gather/aggregate pass ----
            def emit_pass(meta, idx_h, dl_h, src_tab_fn, close_block, pname,
                          extra=None, mid=None):
                nch = meta['nch']
                gb_cur = [-1] * nch
                ig_t = [dict() for _ in range(nch)]
                msg_t = [None] * nch
                cb_cur, oh_t = [-1], [None]
                dl_cur, dl_t = [-1], [None]

                def load_slab(k, ig):
                    t = idx_p.tile([128, IDXG * GB * 8], i16, tag="idx",
                                   name=f"idx{pname}_{k}_{ig}")
                    icol = (int(meta['batch_base'][k]) + ig * IDXG) * GB * 8
                    nb_k = int(meta['nb_per_chunk'][k])
                    nld = min(IDXG, nb_k - ig * IDXG) * GB * 8
                    nc.sync.dma_start(t[:, :nld], idx_h[:, icol:icol + nld])
                    ig_t[k][ig] = t
                for j in range(meta['nblk512']):
                    c0 = j * SWBLK
                    C = min(SWBLK, meta['S'] - c0)
                    nb_seen = 0
                    mid_fn = mid.get(j) if mid else None
                    ps0 = ps0_p.tile([128, SWBLK], f32, tag="ps0")
                    for off in meta['empty_w'][j]:
                        nc.tensor.matmul(ps0[:, off:off + WBLK], zc1[:], zr64[:],
                                         start=True, stop=True)
                    t0, t1 = int(meta['blk_start'][j]), int(meta['blk_end'][j])
                    for t in range(t0, t1):
                        k = int(meta['tile_chunk'][t])
                        gb = int(meta['tile_gb'][t])
                        if gb != gb_cur[k]:
                            gb_cur[k] = gb
                            ig = gb // IDXG
                            nb_k = int(meta['nb_per_chunk'][k])
                            if ig not in ig_t[k]:
                                load_slab(k, ig)
                            if gb % IDXG == 0 and (ig + 1) * IDXG < nb_k \
                                    and (ig + 1) not in ig_t[k]:
                                load_slab(k, ig + 1)
                            if ig - 1 in ig_t[k]:
                                del ig_t[k][ig - 1]
                            msg_t[k] = msg_p.tile([128, GB, 128], bf16, tag="msg",
                                                  name=f"msg{pname}_{k}_{gb}")
                            goff = (gb % IDXG) * GB * 8
                            tab, kbase, kend = src_tab_fn(k)
                            nreal = meta['batch_sizes'][k][gb]
                            gcount[0] += 1
                            if gcount[0] <= MSGB:
                                # first pass through the msg pool: gather full
                                # batches so every buffer lane is initialized
                                # (later trims may leave stale lanes; they are
                                # zeroed by the one-hot, so must be finite)
                                nidx = GB * 128
                                nreal = GB
                            else:
                                nidx = int(meta['batch_nidx'][k][gb])
                            _gi = nc.gpsimd.dma_gather(msg_t[k][:, :nreal, :],
                                                       tab[kbase:kend, :],
                                                       ig_t[k][ig][:, goff:goff + GB * 8],
                                                       nidx, nreg(nidx), 128,
                                                       queue_num=gcount[0] % NSWQ)
                            if os.environ.get("KERNEL_FOLLOW") and pname == "pa":
                                if 70 <= gcount[0] <= 78:
                                    tile.tile_follow(_gi, log_all_deps=True)
                            nb_seen += 1
                            if mid_fn is not None and nb_seen == 8:
                                mid_fn()
                                mid_fn = None
                        cb = t // CBLK
                        if cb != cb_cur[0]:
                            cb_cur[0] = cb
                            dg = t // DLG
                            if dg != dl_cur[0]:
                                dl_cur[0] = dg
                                dl_t[0] = dl_p.tile([128, DLG], bf16, tag="dl",
                                                    name=f"dl{pname}_{dg}")
                                nc.scalar.dma_start(dl_t[0][:],
                                                    dl_h[:, dg * DLG:(dg + 1) * DLG])
                            oh_t[0] = oh_p.tile([128, CBLK, WBLK], bf16, tag="oh",
                                                name=f"oh{pname}_{cb}")
                            dcol = (cb * CBLK) % DLG
                            in0 = dl_t[0][:, dcol:dcol + CBLK].to_broadcast(
                                [128, CBLK, WBLK])
                            _i = iota_f[:]
                            in1 = bass.AP(_i.tensor, _i.offset,
                                          [list(_i.ap[0]), [0, CBLK], list(_i.ap[1])])
                            nc.vector.tensor_tensor(out=oh_t[0][:], in0=in0, in1=in1,
                                                    op=mybir.AluOpType.is_equal)
                        off = int(meta['tile_off'][t])
                        nc.tensor.matmul(ps0[:, off:off + WBLK],
                                         msg_t[k][:, int(meta['tile_gslot'][t]), :],
                                         oh_t[0][:, t % CBLK, :],
                                         start=bool(meta['w_start'][t]),
                                         stop=bool(meta['w_stop'][t]))
                    close_block(j, c0, C, ps0)
                    if extra is not None:
                        extra(j)

            # ---- PA pass: papers -> authors, produces a1 into cat tables ----
            def close_pa(j, c0, C, ps0):
                inv_t = inv_p.tile([64, SWBLK], bf16, tag="inv64")
                nc.scalar.dma_start(inv_t[:, :C], invA_h[:, c0:c0 + C])
                meanT = mean_p.tile([64, SWBLK], bf16, tag="mean64")
                nc.vector.tensor_tensor(out=meanT[:, :C], in0=ps0[0:64, :C],
                                        in1=inv_t[:, :C], op=mybir.AluOpType.mult)
                aT_t = x65_p.tile([65, SWBLK], bf16, tag="x65")
                nc.scalar.dma_start(aT_t[:, :C], aT65_h[:, c0:c0 + C])
                for s in range(0, C, 128):
                    ws = min(128, C - s)
                    psA = psA_p.tile([128, 64], f32, tag="psA")
                    nc.tensor.matmul(psA[:ws, :], meanT[:, s:s + ws], wt["W1l_pa"][:],
                                     start=True, stop=False)
                    nc.tensor.matmul(psA[:ws, :], aT_t[:, s:s + ws], wt["W1rb_pa"][:],
                                     start=False, stop=True)
                    o = sml_p.tile([128, 64], bf16, tag="sml")
                    nc.scalar.activation(o[:ws, :], psA[:ws, :], AF.Relu)
                    nc.sync.dma_start(cat_rows(c0 + s, c0 + s + ws, 1),
                                      o[:ws, :])
                if STAGE < 3:
                    return
                if j == nblkA - 1:
                    if CC4:
                        nc.gpsimd.collective_compute(
                            "AllGather", mybir.AluOpType.bypass, replica_groups=rg,
                            ins=[catB_in_a1[:]], outs=[catB_full[:, 64:128]])
                    else:
                        nc.gpsimd.collective_compute(
                            "AllGather", mybir.AluOpType.bypass, replica_groups=rg,
                            ins=[catB_in[:]], outs=[catB_full[:]])


            def pa_extra(j):
                if j + 2 < nblkA:
                    proj_block(j + 2)
                if CC4 and STAGE >= 3 and j == SPLIT // SWBLK - 2:
                    # proj blocks 0..SPLIT/512-1 all emitted; catA a-half ready
                    nc.gpsimd.collective_compute(
                        "AllGather", mybir.AluOpType.bypass, replica_groups=rg,
                        ins=[catA_in_a[:]], outs=[catA_full[:, 0:64]])
                if CC4 and STAGE >= 3 and j == nblkA - 2:
                    nc.gpsimd.collective_compute(
                        "AllGather", mybir.AluOpType.bypass, replica_groups=rg,
                        ins=[catB_in_a[:]], outs=[catB_full[:, 0:64]])

            def trig_ag_a():
                if STAGE >= 3:
                    if CC4:
                        nc.gpsimd.collective_compute(
                            "AllGather", mybir.AluOpType.bypass, replica_groups=rg,
                            ins=[catA_in_a1[:]], outs=[catA_full[:, 64:128]])
                    else:
                        nc.gpsimd.collective_compute(
                            "AllGather", mybir.AluOpType.bypass, replica_groups=rg,
                            ins=[catA_in[:]], outs=[catA_full[:]])

            if STAGE >= 2:
                emit_pass(mPA, idxPA_h, dlPA_h,
                          lambda k: (p_bf_h, k * 25000, min((k + 1) * 25000, NP_)),
                          close_pa, "pa", extra=pa_extra,
                          mid={SPLIT // SWBLK: trig_ag_a})

            # ---- AP sweep 0: accumulate catA contributions into partials ----
            def close_s0(j, c0, C, ps0):
                nc.scalar.activation(partial[:, c0:c0 + C], ps0[:, :C], AF.Identity)

            if STAGE >= 4:
                emit_pass(mS0, idxS0_h, dlS0_h,
                          lambda k: (catA_full, int(BOUND_S0[k]), int(BOUND_S0[k + 1])),
                          close_s0, "s0")

            # ---- AP sweep 1 + fused conv1/conv2/head epilogue ----
            def close_s1(j, c0, C, ps0):
                sum_t = sum_p.tile([128, SWBLK], f32, tag="sum")
                nc.vector.tensor_tensor(out=sum_t[:, :C], in0=ps0[:, :C],
                                        in1=partial[:, c0:c0 + C],
                                        op=mybir.AluOpType.add)
                inv_t = inv_p.tile([128, SWBLK], bf16, tag="inv128")
                nc.scalar.dma_start(inv_t[:, :C], invP_h[:, c0:c0 + C])
                meanA = mean_p.tile([64, SWBLK], bf16, tag="meanA")
                nc.vector.tensor_tensor(out=meanA[:, :C], in0=sum_t[0:64, :C],
                                        in1=inv_t[0:64, :C], op=mybir.AluOpType.mult)
                meanB = mean_p.tile([64, SWBLK], bf16, tag="meanB")
                nc.vector.tensor_tensor(out=meanB[:, :C], in0=sum_t[64:128, :C],
                                        in1=inv_t[64:128, :C], op=mybir.AluOpType.mult)
                pT_t = x65_p.tile([64, SWBLK], bf16, tag="pTt")
                nc.sync.dma_start(pT_t[:, :C], pT_h[:, c0:c0 + C])
                psE = psE_p.tile([64, SWBLK], f32, tag="psE")
                nc.tensor.matmul(psE[:, :C], wt["W1l_ap"][:], meanA[:, :C],
                                 start=True, stop=False)
                nc.tensor.matmul(psE[:, :C], wt["W1r_ap"][:], pT_t[:, :C],
                                 start=False, stop=True)
                p1 = big_p.tile([64, SWBLK], bf16, tag="big")
                nc.scalar.activation(p1[:, :C], psE[:, :C], AF.Relu, bias=wt["b1c"][:])
                psE2 = psE_p.tile([64, SWBLK], f32, tag="psE")
                nc.tensor.matmul(psE2[:, :C], wt["W2l_ap"][:], meanB[:, :C],
                                 start=True, stop=False)
                nc.tensor.matmul(psE2[:, :C], wt["W2r_ap"][:], p1[:, :C],
                                 start=False, stop=True)
                p2 = big_p.tile([64, SWBLK], bf16, tag="big")
                nc.scalar.activation(p2[:, :C], psE2[:, :C], AF.Relu, bias=wt["b2c"][:])
                psH = psE_p.tile([64, SWBLK], f32, tag="psE")
                nc.tensor.matmul(psH[:, :C], wt["Wl1"][:], p2[:, :C],
                                 start=True, stop=True)
                h = big_p.tile([64, SWBLK], bf16, tag="big")
                nc.scalar.activation(h[:, :C], psH[:, :C], AF.Relu, bias=wt["bl1c"][:])
                psO = psO_p.tile([1, SWBLK], f32, tag="psO")
                nc.tensor.matmul(psO[:, :C], wt["Wl2c"][:], h[:, :C],
                                 start=True, stop=True)
                o = outr_p.tile([1, SWBLK], f32, tag="outrow")
                nc.scalar.activation(o[:, :C], psO[:, :C], AF.Identity,
                                     bias=wt["bl2c"][:])
                nc.sync.dma_start(out_h[:, c0:c0 + C], o[:, :C])

            if STAGE >= 5:
                emit_pass(mS1, idxS1_h, dlS1_h,
                          lambda k: (catB_full, int(BOUND_S1[k]), int(BOUND_S1[k + 1])),
                          close_s1, "s1")

    nc.compile()

    if int(os.environ.get("KERNEL_TLSIM", "1")):
        try:
            from concourse.timeline_sim import TimelineSim
            _t = TimelineSim(nc)
            kernel.modeled_time_ns = _t.simulate()
            print(f"[kernel] TimelineSim modeled core time: "
                  f"{kernel.modeled_time_ns / 1e3:.1f} us")
        except Exception as e:
            print(f"[kernel] TimelineSim failed: {e}")
            kernel.modeled_time_ns = None

    globals()["_last_nc"] = nc
    if int(os.environ.get("KERNEL_BUILD_ONLY", "0")):
        raise SystemExit(0)

    in_maps = []
    for c in range(N_CORES):
        m = {"afT": afT_cores[c], "p_bf": p_bf, "pT": pT_cores[c],
             "invA": invA_cores[c], "invP": invP_cores[c],
             "idxPA": idxPA[c], "dlPA": dlPA[c],
             "idxS0": idxS0[c], "dlS0": dlS0[c],
             "idxS1": idxS1[c], "dlS1": dlS1[c],
             "ones": ones_bf, "iotaf": iota_stage}
        for n, v in zip(wb_names, wb_vals):
            m[n] = v
        for n, v in zip(wf_names, wf_vals):
            m[n] = v
        in_maps.append(m)

    if int(os.environ.get("KERNEL_SIM", "0")):
        from concourse import bass_interp
        sim = bass_interp.MultiCoreSim(nc, N_CORES)
        for c in range(N_CORES):
            for n, v in in_maps[c].items():
                sim.cores[c].tensor(n)[:] = v
        sim.simulate()
        results = [{"out": np.array(sim.cores[c].tensor("out"))} for c in range(N_CORES)]
    else:
        trace = bool(int(os.environ.get("KERNEL_TRACE", "0")))
        res = run_bass_kernel_spmd(nc, in_maps, core_ids=list(range(N_CORES)), trace=trace)
        if trace:
            kernel.last_exec_time_ns = res.exec_time_ns
            kernel.last_results = res
        results = res.results

    out = np.concatenate([results[c]["out"][0] for c in range(N_CORES)])[:NP_]
    return out.reshape(NP_, 1).astype(np.float32)


# revision 67
# speedup vs baseline: 1.5147x; 1.0001x over previous
import sys
import os
sys.path.insert(0, '/opt/trn_rl_repo')
import numpy as np
import ml_dtypes

BF = ml_dtypes.bfloat16

N_CORES = 8
GB = int(os.environ.get("KERNEL_GB", "8"))    # tiles per gather batch
MSGB = int(os.environ.get("KERNEL_MSGB", "16"))  # msg pool buffers
NSWQ = int(os.environ.get("KERNEL_NSWQ", "1"))   # SWDGE queues
WBLK = 64          # one-hot window width
SWBLK = 512        # psum superwindow (one PSUM bank of fp32)
CBLK = 32          # tiles per DVE is_equal batch
DLG = 128          # tiles per dstloc DMA (multiple of CBLK)
IDXG = 4           # gather batches per idx DMA slab
SCRATCH = int(os.environ.get("KERNEL_SCRATCH", "98304"))    # SWDGE ring bytes -> 4096 descriptors (2 batches in flight)

NA, NP_, FIN, H = 100000, 150000, 128, 64
SA, SP = NA // N_CORES, NP_ // N_CORES          # 12500, 18750
SPLIT = 6144                                    # catA rows/core (12 PA blocks)
SPLIT_B = SA - SPLIT                            # 6356
CATA_N = SPLIT * N_CORES                        # 49152
CATB_N = SPLIT_B * N_CORES                      # 50848
BOUND_PA = np.arange(0, NP_ + 24999, 25000)[:7]             # 6 chunks of 25000
BOUND_S0 = np.array([0, CATA_N // 2, CATA_N])               # 2 x 24576
BOUND_S1 = np.array([0, CATB_N // 2, CATB_N])               # 2 x 25424


def _ceil(a, b):
    return -(-a // b)


def _wrap_idx_batch(arr):
    """[GB*128] int16 -> [128, GB*8] staged layout: idx i at (i%16, i//16), x8."""
    n = arr.shape[0]
    w16 = arr.reshape(n // 16, 16).T
    return np.tile(w16, (8, 1))


def _build_pass(dst, src_gid, sel, S, bounds):
    """One gather/aggregate pass: edges [sel], dst sharded by S across cores,
    src gathered from a table addressed by src_gid with chunk boundaries."""
    idx_e = np.nonzero(sel)[0]
    d = dst[idx_e].astype(np.int64)
    s = src_gid[idx_e].astype(np.int64)
    core = d // S
    dst_rel = d - core * S
    nch = len(bounds) - 1
    k_arr = np.searchsorted(bounds, s, side='right') - 1
    src_loc = (s - bounds[k_arr]).astype(np.int16)
    nblk64 = _ceil(S, WBLK)
    nblk512 = _ceil(S, SWBLK)
    m_arr = dst_rel // WBLK
    bucket = m_arr * nch + k_arr
    nbuckets = nblk64 * nch

    flat = core * nbuckets + bucket
    counts = np.bincount(flat, minlength=N_CORES * nbuckets).reshape(N_CORES, nbuckets)
    slots = _ceil(counts, 128).max(axis=0)
    ntiles = int(slots.sum())
    bucket_tile_start = np.zeros(nbuckets + 1, np.int64)
    np.cumsum(slots, out=bucket_tile_start[1:])

    tile_bucket = np.repeat(np.arange(nbuckets), slots)
    tile_m = tile_bucket // nch
    tile_chunk = (tile_bucket % nch).astype(np.int64)
    tile_j = tile_m // (SWBLK // WBLK)
    tile_off = (tile_m % (SWBLK // WBLK)) * WBLK

    blk_start = np.searchsorted(tile_j, np.arange(nblk512), side='left')
    blk_end = np.searchsorted(tile_j, np.arange(nblk512), side='right')

    # per-window psum start/stop flags (window = (j, m) run of tiles)
    w_start = np.ones(ntiles, bool)
    w_stop = np.ones(ntiles, bool)
    if ntiles > 1:
        same = tile_m[1:] == tile_m[:-1]
        w_start[1:] = ~same
        w_stop[:-1] = ~same
    # windows with no tiles at all need explicit psum zeroing
    present = np.zeros(nblk64, bool)
    present[tile_m] = True
    empty_w = [[] for _ in range(nblk512)]
    for m in np.nonzero(~present)[0]:
        empty_w[m // (SWBLK // WBLK)].append(int((m % (SWBLK // WBLK)) * WBLK))

    # max fill (over cores) of each tile; only a bucket's last tile is partial
    tile_fill = np.full(ntiles, 128, np.int64)
    cmax = counts.max(axis=0)
    for b in np.nonzero(slots)[0]:
        last = bucket_tile_start[b + 1] - 1
        tile_fill[last] = min(max(int(cmax[b]) - 128 * (int(slots[b]) - 1), 1), 128)

    # gather batches per chunk
    chunk_tiles = [np.nonzero(tile_chunk == k)[0] for k in range(nch)]
    nb_per_chunk = [_ceil(len(ct), GB) if len(ct) else 0 for ct in chunk_tiles]
    tile_gb = np.zeros(ntiles, np.int64)
    tile_gslot = np.zeros(ntiles, np.int64)
    batch_sizes = []           # [k][b] -> real tiles in batch
    batch_nidx = []            # [k][b] -> num_idxs (last tile trimmed to 16)
    for k in range(nch):
        ct = chunk_tiles[k]
        # within each batch, emptiest tile last -> largest num_idxs trim
        parts = []
        for b in range(nb_per_chunk[k]):
            seg = ct[b * GB:(b + 1) * GB]
            parts.append(seg[np.argsort(-tile_fill[seg], kind='stable')])
        ct = np.concatenate(parts) if parts else ct
        chunk_tiles[k] = ct
        pos = np.arange(len(ct))
        tile_gb[ct] = pos // GB
        tile_gslot[ct] = pos % GB
        bs = [min(GB, len(ct) - b * GB) for b in range(nb_per_chunk[k])]
        batch_sizes.append(bs)
        bn = []
        for b in range(nb_per_chunk[k]):
            tt = ct[b * GB:(b + 1) * GB]
            bn.append((len(tt) - 1) * 128 + _ceil(int(tile_fill[tt[-1]]), 16) * 16)
        batch_nidx.append(bn)
    batch_base = np.zeros(nch + 1, np.int64)
    np.cumsum(nb_per_chunk, out=batch_base[1:])
    NB = max(int(batch_base[-1]), 1)

    ntiles_pad = _ceil(max(ntiles, 1), DLG) * DLG

    order = np.lexsort((bucket, core))
    flat_sorted = flat[order]
    grp_start = np.searchsorted(flat_sorted, flat_sorted, side='left')
    rank = np.arange(len(idx_e)) - grp_start

    idx_staged, dl_staged = [], []
    core_ord = core[order]
    for c in range(N_CORES):
        selc = order[core_ord == c]
        r_sel = rank[core_ord == c]
        b_sel = bucket[selc]
        gtile = bucket_tile_start[b_sel] + r_sel // 128
        lane = r_sel % 128

        idx_flat = np.zeros(ntiles * 128, np.int16)
        idx_flat[gtile * 128 + lane] = src_loc[selc]
        dl = np.full((128, ntiles_pad), -1.0, BF)
        dl[lane, gtile] = (dst_rel[selc] - tile_m[gtile] * WBLK).astype(BF)

        idx_cols = np.zeros((128, NB * GB * 8), np.int16)
        per_tile = idx_flat.reshape(ntiles, 128)
        for k in range(nch):
            ct = chunk_tiles[k]
            for b in range(nb_per_chunk[k]):
                arr = np.zeros(GB * 128, np.int16)
                tt = ct[b * GB:(b + 1) * GB]
                arr[:len(tt) * 128] = per_tile[tt].reshape(-1)
                c0 = (batch_base[k] + b) * GB * 8
                idx_cols[:, c0:c0 + GB * 8] = _wrap_idx_batch(arr)
        idx_staged.append(idx_cols)
        dl_staged.append(dl)

    meta = dict(S=S, nch=nch, bounds=bounds, nblk512=nblk512, ntiles=ntiles,
                ntiles_pad=ntiles_pad, NB=NB, batch_base=batch_base,
                nb_per_chunk=nb_per_chunk, batch_sizes=batch_sizes,
                batch_nidx=batch_nidx,
                tile_chunk=tile_chunk, tile_off=tile_off, tile_gb=tile_gb,
                tile_gslot=tile_gslot, blk_start=blk_start, blk_end=blk_end,
                w_start=w_start, w_stop=w_stop, empty_w=empty_w)
    return meta, idx_staged, dl_staged


def kernel(author_features, edge_author, edge_paper, paper_emb, Wproj, bproj,
           W1l_ap, b1_ap, W1r_ap, W1l_pa, b1_pa, W1r_pa,
           W2l_ap, b2_ap, W2r_ap, W2l_pa, b2_pa, W2r_pa,
           Wl1, bl1, Wl2, bl2):
    import concourse.bass as bass
    import concourse.tile as tile
    from concourse import bacc, mybir
    from concourse.bass_utils import run_bass_kernel_spmd
    from concourse.library_config import mlp

    f32 = mybir.dt.float32
    bf16 = mybir.dt.bfloat16
    i16 = mybir.dt.int16
    AF = mybir.ActivationFunctionType

    af = np.asarray(author_features, np.float32)
    p = np.asarray(paper_emb, np.float32)
    ea = np.asarray(edge_author, np.int64)
    ep = np.asarray(edge_paper, np.int64)
    E = ea.shape[0]

    # ---- host prep: cat-table row mapping for authors ----
    c_a = ea // SA
    r_a = ea - c_a * SA
    gidA = np.where(r_a < SPLIT, c_a * SPLIT + r_a,
                    CATA_N + c_a * SPLIT_B + (r_a - SPLIT))
    selS0 = gidA < CATA_N

    mPA, idxPA, dlPA = _build_pass(ea, ep, np.ones(E, bool), SA, BOUND_PA)
    mS0, idxS0, dlS0 = _build_pass(ep, gidA, selS0, SP, BOUND_S0)
    mS1, idxS1, dlS1 = _build_pass(ep, gidA - CATA_N, ~selS0, SP, BOUND_S1)

    # ---- staged tables ----
    afT_cores = [np.ascontiguousarray(af[c * SA:(c + 1) * SA].T).astype(BF)
                 for c in range(N_CORES)]
    p_bf = np.zeros((NP_, 128), BF)
    p_bf[:, :64] = p.astype(BF)
    pT_cores = [np.ascontiguousarray(p[c * SP:(c + 1) * SP].T).astype(BF)
                for c in range(N_CORES)]

    invA_cores, invP_cores = [], []
    for c in range(N_CORES):
        cntA = np.bincount(ea[ea // SA == c] - c * SA, minlength=SA).astype(np.float32)
        cntP = np.bincount(ep[ep // SP == c] - c * SP, minlength=SP).astype(np.float32)
        invA_cores.append(np.tile((1.0 / np.maximum(cntA, 1.0))[None, :], (64, 1)).astype(BF))
        invP_cores.append(np.tile((1.0 / np.maximum(cntP, 1.0))[None, :], (128, 1)).astype(BF))

    iota_stage = np.tile(np.arange(WBLK, dtype=np.float32)[None, :], (128, 1)).astype(BF)
    ones_bf = np.ones((1, SA), BF)
    W1rb_pa = np.vstack([np.asarray(W1r_pa, np.float32),
                         np.asarray(b1_pa, np.float32)[None]])

    wb_names = ["Wproj", "W1l_ap", "W1r_ap", "W1l_pa", "W1rb_pa",
                "W2l_ap", "W2r_ap", "Wl1", "Wl2c"]
    wb_vals = [np.asarray(Wproj, np.float32).astype(BF),
               np.asarray(W1l_ap, np.float32).astype(BF),
               np.asarray(W1r_ap, np.float32).astype(BF),
               np.asarray(W1l_pa, np.float32).astype(BF),
               W1rb_pa.astype(BF),
               np.asarray(W2l_ap, np.float32).astype(BF),
               np.asarray(W2r_ap, np.float32).astype(BF),
               np.asarray(Wl1, np.float32).astype(BF),
               np.asarray(Wl2, np.float32).reshape(64, 1).astype(BF)]
    # cat 'a' half is stored WITHOUT bproj; fold W1l_ap^T @ bproj into b1
    b1_eff = (np.asarray(b1_ap, np.float32)
              + np.asarray(W1l_ap, np.float32).T @ np.asarray(bproj, np.float32))
    wf_names = ["bprojc", "b1c", "b2c", "bl1c", "bl2c"]
    wf_vals = [np.asarray(bproj, np.float32).reshape(64, 1),
               b1_eff.reshape(64, 1),
               np.asarray(b2_ap, np.float32).reshape(64, 1),
               np.asarray(bl1, np.float32).reshape(64, 1),
               np.asarray(bl2, np.float32).reshape(1, 1)]

    # ---- build program ----
    nc = bacc.Bacc("TRN2", target_bir_lowering=False, debug=False,
                   num_devices=N_CORES, dynamic_dma_scratch_size=SCRATCH,
                   num_swdge_queues=NSWQ)

    afT_h = nc.dram_tensor("afT", [128, SA], bf16, kind="ExternalInput")
    p_bf_h = nc.dram_tensor("p_bf", [NP_, 128], bf16, kind="ExternalInput")
    pT_h = nc.dram_tensor("pT", [64, SP], bf16, kind="ExternalInput")
    invA_h = nc.dram_tensor("invA", [64, SA], bf16, kind="ExternalInput")
    invP_h = nc.dram_tensor("invP", [128, SP], bf16, kind="ExternalInput")
    idxPA_h = nc.dram_tensor("idxPA", list(idxPA[0].shape), i16, kind="ExternalInput")
    dlPA_h = nc.dram_tensor("dlPA", list(dlPA[0].shape), bf16, kind="ExternalInput")
    idxS0_h = nc.dram_tensor("idxS0", list(idxS0[0].shape), i16, kind="ExternalInput")
    dlS0_h = nc.dram_tensor("dlS0", list(dlS0[0].shape), bf16, kind="ExternalInput")
    idxS1_h = nc.dram_tensor("idxS1", list(idxS1[0].shape), i16, kind="ExternalInput")
    dlS1_h = nc.dram_tensor("dlS1", list(dlS1[0].shape), bf16, kind="ExternalInput")
    ones_h = nc.dram_tensor("ones", [1, SA], bf16, kind="ExternalInput")
    iota_h = nc.dram_tensor("iotaf", [128, WBLK], bf16, kind="ExternalInput")
    wb_handles = {n: nc.dram_tensor(n, list(v.shape), bf16, kind="ExternalInput")
                  for n, v in zip(wb_names, wb_vals)}
    wf_handles = {n: nc.dram_tensor(n, list(v.shape), f32, kind="ExternalInput")
                  for n, v in zip(wf_names, wf_vals)}
    out_h = nc.dram_tensor("out", [1, SP], f32, kind="ExternalOutput")

    aT65_h = nc.dram_tensor("aT65", [65, SA], bf16)
    catA_in = nc.dram_tensor("catA_in", [SPLIT, 128], bf16)
    catB_in = nc.dram_tensor("catB_in", [SPLIT_B, 128], bf16)
    catA_in_a = nc.dram_tensor("catA_in_a", [SPLIT, 64], bf16)
    catA_in_a1 = nc.dram_tensor("catA_in_a1", [SPLIT, 64], bf16)
    catB_in_a = nc.dram_tensor("catB_in_a", [SPLIT_B, 64], bf16)
    catB_in_a1 = nc.dram_tensor("catB_in_a1", [SPLIT_B, 64], bf16)
    catA_full = nc.dram_tensor("catA_full", [CATA_N, 128], bf16, addr_space="Shared")
    catB_full = nc.dram_tensor("catB_full", [CATB_N, 128], bf16, addr_space="Shared")

    rg = [list(range(N_CORES))]
    nblkA = SA // SWBLK + (1 if SA % SWBLK else 0)      # 25
    nblkP = mS0['nblk512']                              # 37

    with tile.TileContext(nc) as tc:
        import contextlib
        with contextlib.ExitStack() as ctx:
            const = ctx.enter_context(tc.tile_pool(name="const", bufs=1))
            msg_p = ctx.enter_context(tc.tile_pool(name="msg", bufs=MSGB))
            idx_p = ctx.enter_context(tc.tile_pool(name="idx", bufs=14))
            oh_p = ctx.enter_context(tc.tile_pool(name="oh", bufs=4))
            dl_p = ctx.enter_context(tc.tile_pool(name="dl", bufs=3))
            inv_p = ctx.enter_context(tc.tile_pool(name="inv", bufs=3))
            mean_p = ctx.enter_context(tc.tile_pool(name="mean", bufs=3))
            sum_p = ctx.enter_context(tc.tile_pool(name="sum", bufs=2))
            x65_p = ctx.enter_context(tc.tile_pool(name="x65", bufs=3))
            big_p = ctx.enter_context(tc.tile_pool(name="big", bufs=4))
            sml_p = ctx.enter_context(tc.tile_pool(name="sml", bufs=4))
            aft_p = ctx.enter_context(tc.tile_pool(name="aft", bufs=2))
            outr_p = ctx.enter_context(tc.tile_pool(name="outr", bufs=2))
            ps0_p = ctx.enter_context(tc.tile_pool(name="ps0", bufs=int(os.environ.get("KERNEL_PS0B", "3")), space="PSUM"))
            psE_p = ctx.enter_context(tc.tile_pool(name="psE", bufs=2, space="PSUM"))
            psA_p = ctx.enter_context(tc.tile_pool(name="psA", bufs=2, space="PSUM"))
            psO_p = ctx.enter_context(tc.tile_pool(name="psO", bufs=1, space="PSUM"))

            STAGE = int(os.environ.get("KERNEL_STAGE", "5"))
            CC4 = int(os.environ.get("KERNEL_CC4", "0"))
            nc.gpsimd.load_library(mlp)
            gcount = [0]
            _regs = {}

            def nreg(v):
                if v not in _regs:
                    _regs[v] = nc.gpsimd.to_reg(v)
                return _regs[v]

            nreg(GB * 128)

            wt = {}
            for n, v in zip(wb_names, wb_vals):
                t = const.tile(list(v.shape), bf16, tag=f"w_{n}")
                nc.scalar.dma_start(t[:], wb_handles[n][:])
                wt[n] = t
            for n, v in zip(wf_names, wf_vals):
                t = const.tile(list(v.shape), f32, tag=f"w_{n}")
                nc.scalar.dma_start(t[:], wf_handles[n][:])
                wt[n] = t

            iota_f = const.tile([128, WBLK], bf16, tag="iota_f")
            nc.sync.dma_start(iota_f[:], iota_h[:])
            zc1 = const.tile([1, 128], bf16, tag="zc1")
            nc.vector.memset(zc1[:], 0.0)
            zr64 = const.tile([1, WBLK], bf16, tag="zr64")
            nc.vector.memset(zr64[:], 0.0)
            nc.sync.dma_start(aT65_h[64:65, :], ones_h[:])
            partial = const.tile([128, nblkP * SWBLK], bf16, tag="partial")

            def cat_rows(r0, r1, half):
                """Map local author rows [r0,r1) to (write AP)."""
                if r1 <= SPLIT:
                    if CC4:
                        t = catA_in_a if half == 0 else catA_in_a1
                        return t[r0:r1, :]
                    return catA_in[r0:r1, half * 64:half * 64 + 64]
                assert r0 >= SPLIT
                r0, r1 = r0 - SPLIT, r1 - SPLIT
                if CC4:
                    t = catB_in_a if half == 0 else catB_in_a1
                    return t[r0:r1, :]
                return catB_in[r0:r1, half * 64:half * 64 + 64]

            # ---- projection block: a = af @ Wproj (+bproj only in aT65) ----
            def proj_block(b):
                c0 = b * SWBLK
                cw = min(SWBLK, SA - c0)
                afT_t = aft_p.tile([128, SWBLK], bf16, tag="aft", name=f"afT_{b}")
                nc.sync.dma_start(afT_t[:, :cw], afT_h[:, c0:c0 + cw])
                ps = psE_p.tile([64, SWBLK], f32, tag="psE")
                nc.tensor.matmul(ps[:, :cw], wt["Wproj"][:], afT_t[:, :cw],
                                 start=True, stop=True)
                aTw = big_p.tile([64, SWBLK], bf16, tag="big")
                nc.scalar.activation(aTw[:, :cw], ps[:, :cw], AF.Identity,
                                     bias=wt["bprojc"][:])
                nc.sync.dma_start(aT65_h[0:64, c0:c0 + cw], aTw[:, :cw])
                for s in range(0, cw, 128):
                    ws = min(128, cw - s)
                    psr = psA_p.tile([128, 64], f32, tag="psA")
                    nc.tensor.matmul(psr[:ws, :], afT_t[:, s:s + ws], wt["Wproj"][:],
                                     start=True, stop=True)
                    art = sml_p.tile([128, 64], bf16, tag="sml")
                    nc.scalar.activation(art[:ws, :], psr[:ws, :], AF.Identity)
                    nc.sync.dma_start(cat_rows(c0 + s, c0 + s + ws, 0),
                                      art[:ws, :])

            proj_block(0)
            proj_block(1)

            # ---- generic gather/aggregate pass ----
            def emit_pass(meta, idx_h, dl_h, src_tab_fn, close_block, pname,
                          extra=None, mid=None):
                nch = meta['nch']
                gb_cur = [-1] * nch
                ig_t = [dict() for _ in range(nch)]
                msg_t = [None] * nch
                cb_cur, oh_t = [-1], [None]
                dl_cur, dl_t = [-1], [None]

                def load_slab(k, ig):
                    t = idx_p.tile([128, IDXG * GB * 8], i16, tag="idx",
                                   name=f"idx{pname}_{k}_{ig}")
                    icol = (int(meta['batch_base'][k]) + ig * IDXG) * GB * 8
                    nb_k = int(meta['nb_per_chunk'][k])
                    nld = min(IDXG, nb_k - ig * IDXG) * GB * 8
                    nc.sync.dma_start(t[:, :nld], idx_h[:, icol:icol + nld])
                    ig_t[k][ig] = t
                for j in range(meta['nblk512']):
                    c0 = j * SWBLK
                    C = min(SWBLK, meta['S'] - c0)
                    nb_seen = 0
                    mid_fn = mid.get(j) if mid else None
                    ps0 = ps0_p.tile([128, SWBLK], f32, tag="ps0")
                    for off in meta['empty_w'][j]:
                        nc.tensor.matmul(ps0[:, off:off + WBLK], zc1[:], zr64[:],
                                         start=True, stop=True)
                    t0, t1 = int(meta['blk_start'][j]), int(meta['blk_end'][j])
                    for t in range(t0, t1):
                        k = int(meta['tile_chunk'][t])
                        gb = int(meta['tile_gb'][t])
                        if gb != gb_cur[k]:
                            gb_cur[k] = gb
                            ig = gb // IDXG
                            nb_k = int(meta['nb_per_chunk'][k])
                            if ig not in ig_t[k]:
                                load_slab(k, ig)
                            if gb % IDXG == 0 and (ig + 1) * IDXG < nb_k \
                                    and (ig + 1) not in ig_t[k]:
                                load_slab(k, ig + 1)
                            if ig - 1 in ig_t[k]:
                                del ig_t[k][ig - 1]
                            msg_t[k] = msg_p.tile([128, GB, 128], bf16, tag="msg",
                                                  name=f"msg{pname}_{k}_{gb}")
                            goff = (gb % IDXG) * GB * 8
                            tab, kbase, kend = src_tab_fn(k)
                            nreal = meta['batch_sizes'][k][gb]
                            gcount[0] += 1
                            if gcount[0] <= MSGB:
                                # first pass through the msg pool: gather full
                                # batches so every buffer lane is initialized
                                # (later trims may leave stale lanes; they are
                                # zeroed by the one-hot, so must be finite)
                                nidx = GB * 128
                                nreal = GB
                            else:
                                nidx = int(meta['batch_nidx'][k][gb])
                            _gi = nc.gpsimd.dma_gather(msg_t[k][:, :nreal, :],
                                                       tab[kbase:kend, :],
                                                       ig_t[k][ig][:, goff:goff + GB * 8],
                                                       nidx, nreg(nidx), 128,
                                                       queue_num=gcount[0] % NSWQ)
                            if os.environ.get("KERNEL_FOLLOW") and pname == "pa":
                                if 70 <= gcount[0] <= 78:
                                    tile.tile_follow(_gi, log_all_deps=True)
                            nb_seen += 1
                            if mid_fn is not None and nb_seen == 8:
                                mid_fn()
                                mid_fn = None
                        cb = t // CBLK
                        if cb != cb_cur[0]:
                            cb_cur[0] = cb
                            dg = t // DLG
                            if dg != dl_cur[0]:
                                dl_cur[0] = dg
                                dl_t[0] = dl_p.tile([128, DLG], bf16, tag="dl",
                                                    name=f"dl{pname}_{dg}")
                                nc.scalar.dma_start(dl_t[0][:],
                                                    dl_h[:, dg * DLG:(dg + 1) * DLG])
                            oh_t[0] = oh_p.tile([128, CBLK, WBLK], bf16, tag="oh",
                                                name=f"oh{pname}_{cb}")
                            dcol = (cb * CBLK) % DLG
                            in0 = dl_t[0][:, dcol:dcol + CBLK].to_broadcast(
                                [128, CBLK, WBLK])
                            _i = iota_f[:]
                            in1 = bass.AP(_i.tensor, _i.offset,
                                          [list(_i.ap[0]), [0, CBLK], list(_i.ap[1])])
                            nc.vector.tensor_tensor(out=oh_t[0][:], in0=in0, in1=in1,
                                                    op=mybir.AluOpType.is_equal)
                        off = int(meta['tile_off'][t])
                        nc.tensor.matmul(ps0[:, off:off + WBLK],
                                         msg_t[k][:, int(meta['tile_gslot'][t]), :],
                                         oh_t[0][:, t % CBLK, :],
                                         start=bool(meta['w_start'][t]),
                                         stop=bool(meta['w_stop'][t]))
                    close_block(j, c0, C, ps0)
                    if extra is not None:
                        extra(j)

            # ---- PA pass: papers -> authors, produces a1 into cat tables ----
            def close_pa(j, c0, C, ps0):
                inv_t = inv_p.tile([64, SWBLK], bf16, tag="inv64")
                nc.scalar.dma_start(inv_t[:, :C], invA_h[:, c0:c0 + C])
                meanT = mean_p.tile([64, SWBLK], bf16, tag="mean64")
                nc.vector.tensor_tensor(out=meanT[:, :C], in0=ps0[0:64, :C],
                                        in1=inv_t[:, :C], op=mybir.AluOpType.mult)
                aT_t = x65_p.tile([65, SWBLK], bf16, tag="x65")
                nc.scalar.dma_start(aT_t[:, :C], aT65_h[:, c0:c0 + C])
                for s in range(0, C, 128):
                    ws = min(128, C - s)
                    psA = psA_p.tile([128, 64], f32, tag="psA")
                    nc.tensor.matmul(psA[:ws, :], meanT[:, s:s + ws], wt["W1l_pa"][:],
                                     start=True, stop=False)
                    nc.tensor.matmul(psA[:ws, :], aT_t[:, s:s + ws], wt["W1rb_pa"][:],
                                     start=False, stop=True)
                    o = sml_p.tile([128, 64], bf16, tag="sml")
                    nc.scalar.activation(o[:ws, :], psA[:ws, :], AF.Relu)
                    nc.sync.dma_start(cat_rows(c0 + s, c0 + s + ws, 1),
                                      o[:ws, :])
                if STAGE < 3:
                    return
                if j == nblkA - 1:
                    if CC4:
                        nc.gpsimd.collective_compute(
                            "AllGather", mybir.AluOpType.bypass, replica_groups=rg,
                            ins=[catB_in_a1[:]], outs=[catB_full[:, 64:128]])
                    else:
                        nc.gpsimd.collective_compute(
                            "AllGather", mybir.AluOpType.bypass, replica_groups=rg,
                            ins=[catB_in[:]], outs=[catB_full[:]])


            def pa_extra(j):
                if j + 2 < nblkA:
                    proj_block(j + 2)
                if CC4 and STAGE >= 3 and j == SPLIT // SWBLK - 2:
                    # proj blocks 0..SPLIT/512-1 all emitted; catA a-half ready
                    nc.gpsimd.collective_compute(
                        "AllGather", mybir.AluOpType.bypass, replica_groups=rg,
                        ins=[catA_in_a[:]], outs=[catA_full[:, 0:64]])
                if CC4 and STAGE >= 3 and j == nblkA - 2:
                    nc.gpsimd.collective_compute(
                        "AllGather", mybir.AluOpType.bypass, replica_groups=rg,
                        ins=[catB_in_a[:]], outs=[catB_full[:, 0:64]])

            def trig_ag_a():
                if STAGE >= 3:
                    if CC4:
                        nc.gpsimd.collective_compute(
                            "AllGather", mybir.AluOpType.bypass, replica_groups=rg,
                            ins=[catA_in_a1[:]], outs=[catA_full[:, 64:128]])
                    else:
                        nc.gpsimd.collective_compute(
                            "AllGather", mybir.AluOpType.bypass, replica_groups=rg,
                            ins=[catA_in[:]], outs=[catA_full[:]])

            if STAGE >= 2:
                emit_pass(mPA, idxPA_h, dlPA_h,
                          lambda k: (p_bf_h, k * 25000, min((k + 1) * 25000, NP_)),
                          close_pa, "pa", extra=pa_extra,
                          mid={SPLIT // SWBLK: trig_ag_a})

            # ---- AP sweep 0: accumulate catA contributions into partials ----
            def close_s0(j, c0, C, ps0):
                nc.scalar.activation(partial[:, c0:c0 + C], ps0[:, :C], AF.Identity)

            if STAGE >= 4:
                emit_pass(mS0, idxS0_h, dlS0_h,
                          lambda k: (catA_full, int(BOUND_S0[k]), int(BOUND_S0[k + 1])),
                          close_s0, "s0")

            # ---- AP sweep 1 + fused conv1/conv2/head epilogue ----
            def close_s1(j, c0, C, ps0):
                sum_t = sum_p.tile([128, SWBLK], f32, tag="sum")
                nc.vector.tensor_tensor(out=sum_t[:, :C], in0=ps0[:, :C],
                                        in1=partial[:, c0:c0 + C],
                                        op=mybir.AluOpType.add)
                inv_t = inv_p.tile([128, SWBLK], bf16, tag="inv128")
                nc.scalar.dma_start(inv_t[:, :C], invP_h[:, c0:c0 + C])
                meanA = mean_p.tile([64, SWBLK], bf16, tag="meanA")
                nc.vector.tensor_tensor(out=meanA[:, :C], in0=sum_t[0:64, :C],
                                        in1=inv_t[0:64, :C], op=mybir.AluOpType.mult)
                meanB = mean_p.tile([64, SWBLK], bf16, tag="meanB")
                nc.vector.tensor_tensor(out=meanB[:, :C], in0=sum_t[64:128, :C],
                                        in1=inv_t[64:128, :C], op=mybir.AluOpType.mult)
                pT_t = x65_p.tile([64, SWBLK], bf16, tag="pTt")
                nc.sync.dma_start(pT_t[:, :C], pT_h[:, c0:c0 + C])
                psE = psE_p.tile([64, SWBLK], f32, tag="psE")
                nc.tensor.matmul(psE[:, :C], wt["W1l_ap"][:], meanA[:, :C],
                                 start=True, stop=False)
                nc.tensor.matmul(psE[:, :C], wt["W1r_ap"][:], pT_t[:, :C],
                                 start=False, stop=True)
                p1 = big_p.tile([64, SWBLK], bf16, tag="big")
                nc.scalar.activation(p1[:, :C], psE[:, :C], AF.Relu, bias=wt["b1c"][:])
                psE2 = psE_p.tile([64, SWBLK], f32, tag="psE")
                nc.tensor.matmul(psE2[:, :C], wt["W2l_ap"][:], meanB[:, :C],
                                 start=True, stop=False)
                nc.tensor.matmul(psE2[:, :C], wt["W2r_ap"][:], p1[:, :C],
                                 start=False, stop=True)
                p2 = big_p.tile([64, SWBLK], bf16, tag="big")
                nc.scalar.activation(p2[:, :C], psE2[:, :C], AF.Relu, bias=wt["b2c"][:])
                psH = psE_p.tile([64, SWBLK], f32, tag="psE")
                nc.tensor.matmul(psH[:, :C], wt["Wl1"][:], p2[:, :C],
                                 start=True, stop=True)
                h = big_p.tile([64, SWBLK], bf16, tag="big")
                nc.scalar.activation(h[:, :C], psH[:, :C], AF.Relu, bias=wt["bl1c"][:])
                psO = psO_p.tile([1, SWBLK], f32, tag="psO")
                nc.tensor.matmul(psO[:, :C], wt["Wl2c"][:], h[:, :C],
                                 start=True, stop=True)
                o = outr_p.tile([1, SWBLK], f32, tag="outrow")
                nc.scalar.activation(o[:, :C], psO[:, :C], AF.Identity,
                                     bias=wt["bl2c"][:])
                nc.sync.dma_start(out_h[:, c0:c0 + C], o[:, :C])

            if STAGE >= 5:
                emit_pass(mS1, idxS1_h, dlS1_h,
                          lambda k: (catB_full, int(BOUND_S1[k]), int(BOUND_S1[k + 1])),
                          close_s1, "s1")

    nc.compile()

    if int(os.environ.get("KERNEL_TLSIM", "1")):
        try:
            from concourse.timeline_sim import TimelineSim
            _t = TimelineSim(nc)
            kernel.modeled_time_ns = _t.simulate()
            print(f"[kernel] TimelineSim modeled core time: "
                  f"{kernel.modeled_time_ns / 1e3:.1f} us")
        except Exception as e:
            print(f"[kernel] TimelineSim failed: {e}")
            kernel.modeled_time_ns = None

    globals()["_last_nc"] = nc
    if int(os.environ.get("KERNEL_BUILD_ONLY", "0")):
        raise SystemExit(0)

    in_maps = []
    for c in range(N_CORES):
        m = {"afT": afT_cores[c], "p_bf": p_bf, "pT": pT_cores[c],
             "invA": invA_cores[c], "invP": invP_cores[c],
             "idxPA": idxPA[c], "dlPA": dlPA[c],
             "idxS0": idxS0[c], "dlS0": dlS0[c],
             "idxS1": idxS1[c], "dlS1": dlS1[c],
             "ones": ones_bf, "iotaf": iota_stage}
        for n, v in zip(wb_names, wb_vals):
            m[n] = v
        for n, v in zip(wf_names, wf_vals):
            m[n] = v
        in_maps.append(m)

    if int(os.environ.get("KERNEL_SIM", "0")):
        from concourse import bass_interp
        sim = bass_interp.MultiCoreSim(nc, N_CORES)
        for c in range(N_CORES):
            for n, v in in_maps[c].items():
                sim.cores[c].tensor(n)[:] = v
        sim.simulate()
        results = [{"out": np.array(sim.cores[c].tensor("out"))} for c in range(N_CORES)]
    else:
        trace = bool(int(os.environ.get("KERNEL_TRACE", "0")))
        res = run_bass_kernel_spmd(nc, in_maps, core_ids=list(range(N_CORES)), trace=trace)
        if trace:
            kernel.last_exec_time_ns = res.exec_time_ns
            kernel.last_results = res
        results = res.results

    out = np.concatenate([results[c]["out"][0] for c in range(N_CORES)])[:NP_]
    return out.reshape(NP_, 1).astype(np.float32)


# revision 68
# speedup vs baseline: 1.5159x; 1.0008x over previous
import sys
import os
sys.path.insert(0, '/opt/trn_rl_repo')
import numpy as np
import ml_dtypes

BF = ml_dtypes.bfloat16

N_CORES = 8
GB = int(os.environ.get("KERNEL_GB", "8"))    # tiles per gather batch
MSGB = int(os.environ.get("KERNEL_MSGB", "16"))  # msg pool buffers
NSWQ = int(os.environ.get("KERNEL_NSWQ", "1"))   # SWDGE queues
WBLK = 64          # one-hot window width
SWBLK = 512        # psum superwindow (one PSUM bank of fp32)
CBLK = 32          # tiles per DVE is_equal batch
DLG = 128          # tiles per dstloc DMA (multiple of CBLK)
IDXG = 4           # gather batches per idx DMA slab
SCRATCH = int(os.environ.get("KERNEL_SCRATCH", "98304"))    # SWDGE ring bytes -> 4096 descriptors (2 batches in flight)

NA, NP_, FIN, H = 100000, 150000, 128, 64
SA, SP = NA // N_CORES, NP_ // N_CORES          # 12500, 18750
SPLIT = 6144                                    # catA rows/core (12 PA blocks)
SPLIT_B = SA - SPLIT                            # 6356
CATA_N = SPLIT * N_CORES                        # 49152
CATB_N = SPLIT_B * N_CORES                      # 50848
BOUND_PA = np.arange(0, NP_ + 24999, 25000)[:7]             # 6 chunks of 25000
BOUND_S0 = np.array([0, CATA_N // 2, CATA_N])               # 2 x 24576
BOUND_S1 = np.array([0, CATB_N // 2, CATB_N])               # 2 x 25424


def _ceil(a, b):
    return -(-a // b)


def _wrap_idx_batch(arr):
    """[GB*128] int16 -> [128, GB*8] staged layout: idx i at (i%16, i//16), x8."""
    n = arr.shape[0]
    w16 = arr.reshape(n // 16, 16).T
    return np.tile(w16, (8, 1))


def _build_pass(dst, src_gid, sel, S, bounds):
    """One gather/aggregate pass: edges [sel], dst sharded by S across cores,
    src gathered from a table addressed by src_gid with chunk boundaries."""
    idx_e = np.nonzero(sel)[0]
    d = dst[idx_e].astype(np.int64)
    s = src_gid[idx_e].astype(np.int64)
    core = d // S
    dst_rel = d - core * S
    nch = len(bounds) - 1
    k_arr = np.searchsorted(bounds, s, side='right') - 1
    src_loc = (s - bounds[k_arr]).astype(np.int16)
    nblk64 = _ceil(S, WBLK)
    nblk512 = _ceil(S, SWBLK)
    m_arr = dst_rel // WBLK
    bucket = m_arr * nch + k_arr
    nbuckets = nblk64 * nch

    flat = core * nbuckets + bucket
    counts = np.bincount(flat, minlength=N_CORES * nbuckets).reshape(N_CORES, nbuckets)
    slots = _ceil(counts, 128).max(axis=0)
    ntiles = int(slots.sum())
    bucket_tile_start = np.zeros(nbuckets + 1, np.int64)
    np.cumsum(slots, out=bucket_tile_start[1:])

    tile_bucket = np.repeat(np.arange(nbuckets), slots)
    tile_m = tile_bucket // nch
    tile_chunk = (tile_bucket % nch).astype(np.int64)
    tile_j = tile_m // (SWBLK // WBLK)
    tile_off = (tile_m % (SWBLK // WBLK)) * WBLK

    blk_start = np.searchsorted(tile_j, np.arange(nblk512), side='left')
    blk_end = np.searchsorted(tile_j, np.arange(nblk512), side='right')

    # per-window psum start/stop flags (window = (j, m) run of tiles)
    w_start = np.ones(ntiles, bool)
    w_stop = np.ones(ntiles, bool)
    if ntiles > 1:
        same = tile_m[1:] == tile_m[:-1]
        w_start[1:] = ~same
        w_stop[:-1] = ~same
    # windows with no tiles at all need explicit psum zeroing
    present = np.zeros(nblk64, bool)
    present[tile_m] = True
    empty_w = [[] for _ in range(nblk512)]
    for m in np.nonzero(~present)[0]:
        empty_w[m // (SWBLK // WBLK)].append(int((m % (SWBLK // WBLK)) * WBLK))

    # max fill (over cores) of each tile; only a bucket's last tile is partial
    tile_fill = np.full(ntiles, 128, np.int64)
    cmax = counts.max(axis=0)
    for b in np.nonzero(slots)[0]:
        last = bucket_tile_start[b + 1] - 1
        tile_fill[last] = min(max(int(cmax[b]) - 128 * (int(slots[b]) - 1), 1), 128)

    # gather batches per chunk
    chunk_tiles = [np.nonzero(tile_chunk == k)[0] for k in range(nch)]
    nb_per_chunk = [_ceil(len(ct), GB) if len(ct) else 0 for ct in chunk_tiles]
    tile_gb = np.zeros(ntiles, np.int64)
    tile_gslot = np.zeros(ntiles, np.int64)
    batch_sizes = []           # [k][b] -> real tiles in batch
    batch_nidx = []            # [k][b] -> num_idxs (last tile trimmed to 16)
    for k in range(nch):
        ct = chunk_tiles[k]
        # within each batch, emptiest tile last -> largest num_idxs trim
        parts = []
        for b in range(nb_per_chunk[k]):
            seg = ct[b * GB:(b + 1) * GB]
            parts.append(seg[np.argsort(-tile_fill[seg], kind='stable')])
        ct = np.concatenate(parts) if parts else ct
        chunk_tiles[k] = ct
        pos = np.arange(len(ct))
        tile_gb[ct] = pos // GB
        tile_gslot[ct] = pos % GB
        bs = [min(GB, len(ct) - b * GB) for b in range(nb_per_chunk[k])]
        batch_sizes.append(bs)
        bn = []
        for b in range(nb_per_chunk[k]):
            tt = ct[b * GB:(b + 1) * GB]
            bn.append((len(tt) - 1) * 128 + _ceil(int(tile_fill[tt[-1]]), 16) * 16)
        batch_nidx.append(bn)
    batch_base = np.zeros(nch + 1, np.int64)
    np.cumsum(nb_per_chunk, out=batch_base[1:])
    NB = max(int(batch_base[-1]), 1)

    ntiles_pad = _ceil(max(ntiles, 1), DLG) * DLG

    order = np.lexsort((bucket, core))
    flat_sorted = flat[order]
    grp_start = np.searchsorted(flat_sorted, flat_sorted, side='left')
    rank = np.arange(len(idx_e)) - grp_start

    idx_staged, dl_staged = [], []
    core_ord = core[order]
    for c in range(N_CORES):
        selc = order[core_ord == c]
        r_sel = rank[core_ord == c]
        b_sel = bucket[selc]
        gtile = bucket_tile_start[b_sel] + r_sel // 128
        lane = r_sel % 128

        idx_flat = np.zeros(ntiles * 128, np.int16)
        idx_flat[gtile * 128 + lane] = src_loc[selc]
        dl = np.full((128, ntiles_pad), -1.0, BF)
        dl[lane, gtile] = (dst_rel[selc] - tile_m[gtile] * WBLK).astype(BF)

        idx_cols = np.zeros((128, NB * GB * 8), np.int16)
        per_tile = idx_flat.reshape(ntiles, 128)
        for k in range(nch):
            ct = chunk_tiles[k]
            for b in range(nb_per_chunk[k]):
                arr = np.zeros(GB * 128, np.int16)
                tt = ct[b * GB:(b + 1) * GB]
                arr[:len(tt) * 128] = per_tile[tt].reshape(-1)
                c0 = (batch_base[k] + b) * GB * 8
                idx_cols[:, c0:c0 + GB * 8] = _wrap_idx_batch(arr)
        idx_staged.append(idx_cols)
        dl_staged.append(dl)

    meta = dict(S=S, nch=nch, bounds=bounds, nblk512=nblk512, ntiles=ntiles,
                ntiles_pad=ntiles_pad, NB=NB, batch_base=batch_base,
                nb_per_chunk=nb_per_chunk, batch_sizes=batch_sizes,
                batch_nidx=batch_nidx,
                tile_chunk=tile_chunk, tile_off=tile_off, tile_gb=tile_gb,
                tile_gslot=tile_gslot, blk_start=blk_start, blk_end=blk_end,
                w_start=w_start, w_stop=w_stop, empty_w=empty_w)
    return meta, idx_staged, dl_staged


def kernel(author_features, edge_author, edge_paper, paper_emb, Wproj, bproj,
           W1l_ap, b1_ap, W1r_ap, W1l_pa, b1_pa, W1r_pa,
           W2l_ap, b2_ap, W2r_ap, W2l_pa, b2_pa, W2r_pa,
           Wl1, bl1, Wl2, bl2):
    import concourse.bass as bass
    import concourse.tile as tile
    from concourse import bacc, mybir
    from concourse.bass_utils import run_bass_kernel_spmd
    from concourse.library_config import mlp

    f32 = mybir.dt.float32
    bf16 = mybir.dt.bfloat16
    i16 = mybir.dt.int16
    AF = mybir.ActivationFunctionType

    af = np.asarray(author_features, np.float32)
    p = np.asarray(paper_emb, np.float32)
    ea = np.asarray(edge_author, np.int64)
    ep = np.asarray(edge_paper, np.int64)
    E = ea.shape[0]

    # ---- host prep: cat-table row mapping for authors ----
    c_a = ea // SA
    r_a = ea - c_a * SA
    gidA = np.where(r_a < SPLIT, c_a * SPLIT + r_a,
                    CATA_N + c_a * SPLIT_B + (r_a - SPLIT))
    selS0 = gidA < CATA_N

    mPA, idxPA, dlPA = _build_pass(ea, ep, np.ones(E, bool), SA, BOUND_PA)
    mS0, idxS0, dlS0 = _build_pass(ep, gidA, selS0, SP, BOUND_S0)
    mS1, idxS1, dlS1 = _build_pass(ep, gidA - CATA_N, ~selS0, SP, BOUND_S1)

    # ---- staged tables ----
    afT_cores = [np.ascontiguousarray(af[c * SA:(c + 1) * SA].T).astype(BF)
                 for c in range(N_CORES)]
    p_bf = np.zeros((NP_, 128), BF)
    p_bf[:, :64] = p.astype(BF)
    pT_cores = [np.ascontiguousarray(p[c * SP:(c + 1) * SP].T).astype(BF)
                for c in range(N_CORES)]

    invA_cores, invP_cores = [], []
    for c in range(N_CORES):
        cntA = np.bincount(ea[ea // SA == c] - c * SA, minlength=SA).astype(np.float32)
        cntP = np.bincount(ep[ep // SP == c] - c * SP, minlength=SP).astype(np.float32)
        invA_cores.append(np.tile((1.0 / np.maximum(cntA, 1.0))[None, :], (64, 1)).astype(BF))
        invP_cores.append(np.tile((1.0 / np.maximum(cntP, 1.0))[None, :], (128, 1)).astype(BF))

    iota_stage = np.tile(np.arange(WBLK, dtype=np.float32)[None, :], (128, 1)).astype(BF)
    ones_bf = np.ones((1, SA), BF)
    W1rb_pa = np.vstack([np.asarray(W1r_pa, np.float32),
                         np.asarray(b1_pa, np.float32)[None]])

    wb_names = ["Wproj", "W1l_ap", "W1r_ap", "W1l_pa", "W1rb_pa",
                "W2l_ap", "W2r_ap", "Wl1", "Wl2c"]
    wb_vals = [np.asarray(Wproj, np.float32).astype(BF),
               np.asarray(W1l_ap, np.float32).astype(BF),
               np.asarray(W1r_ap, np.float32).astype(BF),
               np.asarray(W1l_pa, np.float32).astype(BF),
               W1rb_pa.astype(BF),
               np.asarray(W2l_ap, np.float32).astype(BF),
               np.asarray(W2r_ap, np.float32).astype(BF),
               np.asarray(Wl1, np.float32).astype(BF),
               np.asarray(Wl2, np.float32).reshape(64, 1).astype(BF)]
    # cat 'a' half is stored WITHOUT bproj; fold W1l_ap^T @ bproj into b1
    b1_eff = (np.asarray(b1_ap, np.float32)
              + np.asarray(W1l_ap, np.float32).T @ np.asarray(bproj, np.float32))
    wf_names = ["bprojc", "b1c", "b2c", "bl1c", "bl2c"]
    wf_vals = [np.asarray(bproj, np.float32).reshape(64, 1),
               b1_eff.reshape(64, 1),
               np.asarray(b2_ap, np.float32).reshape(64, 1),
               np.asarray(bl1, np.float32).reshape(64, 1),
               np.asarray(bl2, np.float32).reshape(1, 1)]

    # ---- build program ----
    nc = bacc.Bacc("TRN2", target_bir_lowering=False, debug=False,
                   num_devices=N_CORES, dynamic_dma_scratch_size=SCRATCH,
                   num_swdge_queues=NSWQ)

    afT_h = nc.dram_tensor("afT", [128, SA], bf16, kind="ExternalInput")
    p_bf_h = nc.dram_tensor("p_bf", [NP_, 128], bf16, kind="ExternalInput")
    pT_h = nc.dram_tensor("pT", [64, SP], bf16, kind="ExternalInput")
    invA_h = nc.dram_tensor("invA", [64, SA], bf16, kind="ExternalInput")
    invP_h = nc.dram_tensor("invP", [128, SP], bf16, kind="ExternalInput")
    idxPA_h = nc.dram_tensor("idxPA", list(idxPA[0].shape), i16, kind="ExternalInput")
    dlPA_h = nc.dram_tensor("dlPA", list(dlPA[0].shape), bf16, kind="ExternalInput")
    idxS0_h = nc.dram_tensor("idxS0", list(idxS0[0].shape), i16, kind="ExternalInput")
    dlS0_h = nc.dram_tensor("dlS0", list(dlS0[0].shape), bf16, kind="ExternalInput")
    idxS1_h = nc.dram_tensor("idxS1", list(idxS1[0].shape), i16, kind="ExternalInput")
    dlS1_h = nc.dram_tensor("dlS1", list(dlS1[0].shape), bf16, kind="ExternalInput")
    ones_h = nc.dram_tensor("ones", [1, SA], bf16, kind="ExternalInput")
    iota_h = nc.dram_tensor("iotaf", [128, WBLK], bf16, kind="ExternalInput")
    wb_handles = {n: nc.dram_tensor(n, list(v.shape), bf16, kind="ExternalInput")
                  for n, v in zip(wb_names, wb_vals)}
    wf_handles = {n: nc.dram_tensor(n, list(v.shape), f32, kind="ExternalInput")
                  for n, v in zip(wf_names, wf_vals)}
    out_h = nc.dram_tensor("out", [1, SP], f32, kind="ExternalOutput")

    aT65_h = nc.dram_tensor("aT65", [65, SA], bf16)
    catA_in = nc.dram_tensor("catA_in", [SPLIT, 128], bf16)
    catB_in = nc.dram_tensor("catB_in", [SPLIT_B, 128], bf16)
    catA_in_a = nc.dram_tensor("catA_in_a", [SPLIT, 64], bf16)
    catA_in_a1 = nc.dram_tensor("catA_in_a1", [SPLIT, 64], bf16)
    catB_in_a = nc.dram_tensor("catB_in_a", [SPLIT_B, 64], bf16)
    catB_in_a1 = nc.dram_tensor("catB_in_a1", [SPLIT_B, 64], bf16)
    catA_full = nc.dram_tensor("catA_full", [CATA_N, 128], bf16, addr_space="Shared")
    catB_full = nc.dram_tensor("catB_full", [CATB_N, 128], bf16, addr_space="Shared")

    rg = [list(range(N_CORES))]
    nblkA = SA // SWBLK + (1 if SA % SWBLK else 0)      # 25
    nblkP = mS0['nblk512']                              # 37

    with tile.TileContext(nc) as tc:
        import contextlib
        with contextlib.ExitStack() as ctx:
            const = ctx.enter_context(tc.tile_pool(name="const", bufs=1))
            msg_p = ctx.enter_context(tc.tile_pool(name="msg", bufs=MSGB))
            idx_p = ctx.enter_context(tc.tile_pool(name="idx", bufs=14))
            oh_p = ctx.enter_context(tc.tile_pool(name="oh", bufs=4))
            dl_p = ctx.enter_context(tc.tile_pool(name="dl", bufs=3))
            inv_p = ctx.enter_context(tc.tile_pool(name="inv", bufs=3))
            mean_p = ctx.enter_context(tc.tile_pool(name="mean", bufs=3))
            sum_p = ctx.enter_context(tc.tile_pool(name="sum", bufs=2))
            x65_p = ctx.enter_context(tc.tile_pool(name="x65", bufs=3))
            big_p = ctx.enter_context(tc.tile_pool(name="big", bufs=4))
            sml_p = ctx.enter_context(tc.tile_pool(name="sml", bufs=4))
            aft_p = ctx.enter_context(tc.tile_pool(name="aft", bufs=2))
            outr_p = ctx.enter_context(tc.tile_pool(name="outr", bufs=2))
            ps0_p = ctx.enter_context(tc.tile_pool(name="ps0", bufs=int(os.environ.get("KERNEL_PS0B", "3")), space="PSUM"))
            psE_p = ctx.enter_context(tc.tile_pool(name="psE", bufs=2, space="PSUM"))
            psA_p = ctx.enter_context(tc.tile_pool(name="psA", bufs=2, space="PSUM"))
            psO_p = ctx.enter_context(tc.tile_pool(name="psO", bufs=1, space="PSUM"))

            STAGE = int(os.environ.get("KERNEL_STAGE", "5"))
            CC4 = int(os.environ.get("KERNEL_CC4", "0"))
            nc.gpsimd.load_library(mlp)
            gcount = [0]
            _regs = {}

            def nreg(v):
                if v not in _regs:
                    _regs[v] = nc.gpsimd.to_reg(v)
                return _regs[v]

            nreg(GB * 128)

            wt = {}
            for n, v in zip(wb_names, wb_vals):
                t = const.tile(list(v.shape), bf16, tag=f"w_{n}")
                nc.scalar.dma_start(t[:], wb_handles[n][:])
                wt[n] = t
            for n, v in zip(wf_names, wf_vals):
                t = const.tile(list(v.shape), f32, tag=f"w_{n}")
                nc.scalar.dma_start(t[:], wf_handles[n][:])
                wt[n] = t

            iota_f = const.tile([128, WBLK], bf16, tag="iota_f")
            nc.sync.dma_start(iota_f[:], iota_h[:])
            zc1 = const.tile([1, 128], bf16, tag="zc1")
            nc.vector.memset(zc1[:], 0.0)
            zr64 = const.tile([1, WBLK], bf16, tag="zr64")
            nc.vector.memset(zr64[:], 0.0)
            nc.sync.dma_start(aT65_h[64:65, :], ones_h[:])
            partial = const.tile([128, nblkP * SWBLK], bf16, tag="partial")

            def cat_rows(r0, r1, half):
                """Map local author rows [r0,r1) to (write AP)."""
                if r1 <= SPLIT:
                    if CC4:
                        t = catA_in_a if half == 0 else catA_in_a1
                        return t[r0:r1, :]
                    return catA_in[r0:r1, half * 64:half * 64 + 64]
                assert r0 >= SPLIT
                r0, r1 = r0 - SPLIT, r1 - SPLIT
                if CC4:
                    t = catB_in_a if half == 0 else catB_in_a1
                    return t[r0:r1, :]
                return catB_in[r0:r1, half * 64:half * 64 + 64]

            # ---- projection block: a = af @ Wproj (+bproj only in aT65) ----
            def proj_block(b):
                c0 = b * SWBLK
                cw = min(SWBLK, SA - c0)
                afT_t = aft_p.tile([128, SWBLK], bf16, tag="aft", name=f"afT_{b}")
                nc.sync.dma_start(afT_t[:, :cw], afT_h[:, c0:c0 + cw])
                ps = psE_p.tile([64, SWBLK], f32, tag="psE")
                nc.tensor.matmul(ps[:, :cw], wt["Wproj"][:], afT_t[:, :cw],
                                 start=True, stop=True)
                aTw = big_p.tile([64, SWBLK], bf16, tag="big")
                nc.scalar.activation(aTw[:, :cw], ps[:, :cw], AF.Identity,
                                     bias=wt["bprojc"][:])
                nc.sync.dma_start(aT65_h[0:64, c0:c0 + cw], aTw[:, :cw])
                for s in range(0, cw, 128):
                    ws = min(128, cw - s)
                    psr = psA_p.tile([128, 64], f32, tag="psA")
                    nc.tensor.matmul(psr[:ws, :], afT_t[:, s:s + ws], wt["Wproj"][:],
                                     start=True, stop=True)
                    art = sml_p.tile([128, 64], bf16, tag="sml")
                    nc.scalar.activation(art[:ws, :], psr[:ws, :], AF.Identity)
                    nc.sync.dma_start(cat_rows(c0 + s, c0 + s + ws, 0),
                                      art[:ws, :])

            proj_block(0)
            proj_block(1)

            # ---- generic gather/aggregate pass ----
            def emit_pass(meta, idx_h, dl_h, src_tab_fn, close_block, pname,
                          extra=None, mid=None):
                nch = meta['nch']
                gb_cur = [-1] * nch
                ig_t = [dict() for _ in range(nch)]
                msg_t = [None] * nch
                cb_cur, oh_t = [-1], [None]
                dl_cur, dl_t = [-1], [None]

                def load_slab(k, ig):
                    t = idx_p.tile([128, IDXG * GB * 8], i16, tag="idx",
                                   name=f"idx{pname}_{k}_{ig}")
                    icol = (int(meta['batch_base'][k]) + ig * IDXG) * GB * 8
                    nb_k = int(meta['nb_per_chunk'][k])
                    nld = min(IDXG, nb_k - ig * IDXG) * GB * 8
                    nc.sync.dma_start(t[:, :nld], idx_h[:, icol:icol + nld])
                    ig_t[k][ig] = t

                for k in range(nch):
                    if meta['nb_per_chunk'][k]:
                        load_slab(k, 0)
                for j in range(meta['nblk512']):
                    c0 = j * SWBLK
                    C = min(SWBLK, meta['S'] - c0)
                    nb_seen = 0
                    mid_fn = mid.get(j) if mid else None
                    ps0 = ps0_p.tile([128, SWBLK], f32, tag="ps0")
                    for off in meta['empty_w'][j]:
                        nc.tensor.matmul(ps0[:, off:off + WBLK], zc1[:], zr64[:],
                                         start=True, stop=True)
                    t0, t1 = int(meta['blk_start'][j]), int(meta['blk_end'][j])
                    for t in range(t0, t1):
                        k = int(meta['tile_chunk'][t])
                        gb = int(meta['tile_gb'][t])
                        if gb != gb_cur[k]:
                            gb_cur[k] = gb
                            ig = gb // IDXG
                            nb_k = int(meta['nb_per_chunk'][k])
                            if ig not in ig_t[k]:
                                load_slab(k, ig)
                            if gb % IDXG == 0 and (ig + 1) * IDXG < nb_k \
                                    and (ig + 1) not in ig_t[k]:
                                load_slab(k, ig + 1)
                            if ig - 1 in ig_t[k]:
                                del ig_t[k][ig - 1]
                            msg_t[k] = msg_p.tile([128, GB, 128], bf16, tag="msg",
                                                  name=f"msg{pname}_{k}_{gb}")
                            goff = (gb % IDXG) * GB * 8
                            tab, kbase, kend = src_tab_fn(k)
                            nreal = meta['batch_sizes'][k][gb]
                            gcount[0] += 1
                            if gcount[0] <= MSGB:
                                # first pass through the msg pool: gather full
                                # batches so every buffer lane is initialized
                                # (later trims may leave stale lanes; they are
                                # zeroed by the one-hot, so must be finite)
                                nidx = GB * 128
                                nreal = GB
                            else:
                                nidx = int(meta['batch_nidx'][k][gb])
                            _gi = nc.gpsimd.dma_gather(msg_t[k][:, :nreal, :],
                                                       tab[kbase:kend, :],
                                                       ig_t[k][ig][:, goff:goff + GB * 8],
                                                       nidx, nreg(nidx), 128,
                                                       queue_num=gcount[0] % NSWQ)
                            if os.environ.get("KERNEL_FOLLOW") and pname == "pa":
                                if 70 <= gcount[0] <= 78:
                                    tile.tile_follow(_gi, log_all_deps=True)
                            nb_seen += 1
                            if mid_fn is not None and nb_seen == 8:
                                mid_fn()
                                mid_fn = None
                        cb = t // CBLK
                        if cb != cb_cur[0]:
                            cb_cur[0] = cb
                            dg = t // DLG
                            if dg != dl_cur[0]:
                                dl_cur[0] = dg
                                dl_t[0] = dl_p.tile([128, DLG], bf16, tag="dl",
                                                    name=f"dl{pname}_{dg}")
                                nc.scalar.dma_start(dl_t[0][:],
                                                    dl_h[:, dg * DLG:(dg + 1) * DLG])
                            oh_t[0] = oh_p.tile([128, CBLK, WBLK], bf16, tag="oh",
                                                name=f"oh{pname}_{cb}")
                            dcol = (cb * CBLK) % DLG
                            in0 = dl_t[0][:, dcol:dcol + CBLK].to_broadcast(
                                [128, CBLK, WBLK])
                            _i = iota_f[:]
                            in1 = bass.AP(_i.tensor, _i.offset,
                                          [list(_i.ap[0]), [0, CBLK], list(_i.ap[1])])
                            nc.vector.tensor_tensor(out=oh_t[0][:], in0=in0, in1=in1,
                                                    op=mybir.AluOpType.is_equal)
                        off = int(meta['tile_off'][t])
                        nc.tensor.matmul(ps0[:, off:off + WBLK],
                                         msg_t[k][:, int(meta['tile_gslot'][t]), :],
                                         oh_t[0][:, t % CBLK, :],
                                         start=bool(meta['w_start'][t]),
                                         stop=bool(meta['w_stop'][t]))
                    close_block(j, c0, C, ps0)
                    if extra is not None:
                        extra(j)

            # ---- PA pass: papers -> authors, produces a1 into cat tables ----
            def close_pa(j, c0, C, ps0):
                inv_t = inv_p.tile([64, SWBLK], bf16, tag="inv64")
                nc.scalar.dma_start(inv_t[:, :C], invA_h[:, c0:c0 + C])
                meanT = mean_p.tile([64, SWBLK], bf16, tag="mean64")
                nc.vector.tensor_tensor(out=meanT[:, :C], in0=ps0[0:64, :C],
                                        in1=inv_t[:, :C], op=mybir.AluOpType.mult)
                aT_t = x65_p.tile([65, SWBLK], bf16, tag="x65")
                nc.scalar.dma_start(aT_t[:, :C], aT65_h[:, c0:c0 + C])
                for s in range(0, C, 128):
                    ws = min(128, C - s)
                    psA = psA_p.tile([128, 64], f32, tag="psA")
                    nc.tensor.matmul(psA[:ws, :], meanT[:, s:s + ws], wt["W1l_pa"][:],
                                     start=True, stop=False)
                    nc.tensor.matmul(psA[:ws, :], aT_t[:, s:s + ws], wt["W1rb_pa"][:],
                                     start=False, stop=True)
                    o = sml_p.tile([128, 64], bf16, tag="sml")
                    nc.scalar.activation(o[:ws, :], psA[:ws, :], AF.Relu)
                    nc.sync.dma_start(cat_rows(c0 + s, c0 + s + ws, 1),
                                      o[:ws, :])
                if STAGE < 3:
                    return
                if j == nblkA - 1:
                    if CC4:
                        nc.gpsimd.collective_compute(
                            "AllGather", mybir.AluOpType.bypass, replica_groups=rg,
                            ins=[catB_in_a1[:]], outs=[catB_full[:, 64:128]])
                    else:
                        nc.gpsimd.collective_compute(
                            "AllGather", mybir.AluOpType.bypass, replica_groups=rg,
                            ins=[catB_in[:]], outs=[catB_full[:]])


            def pa_extra(j):
                if j + 2 < nblkA:
                    proj_block(j + 2)
                if CC4 and STAGE >= 3 and j == SPLIT // SWBLK - 2:
                    # proj blocks 0..SPLIT/512-1 all emitted; catA a-half ready
                    nc.gpsimd.collective_compute(
                        "AllGather", mybir.AluOpType.bypass, replica_groups=rg,
                        ins=[catA_in_a[:]], outs=[catA_full[:, 0:64]])
                if CC4 and STAGE >= 3 and j == nblkA - 2:
                    nc.gpsimd.collective_compute(
                        "AllGather", mybir.AluOpType.bypass, replica_groups=rg,
                        ins=[catB_in_a[:]], outs=[catB_full[:, 0:64]])

            def trig_ag_a():
                if STAGE >= 3:
                    if CC4:
                        nc.gpsimd.collective_compute(
                            "AllGather", mybir.AluOpType.bypass, replica_groups=rg,
                            ins=[catA_in_a1[:]], outs=[catA_full[:, 64:128]])
                    else:
                        nc.gpsimd.collective_compute(
                            "AllGather", mybir.AluOpType.bypass, replica_groups=rg,
                            ins=[catA_in[:]], outs=[catA_full[:]])

            if STAGE >= 2:
                emit_pass(mPA, idxPA_h, dlPA_h,
                          lambda k: (p_bf_h, k * 25000, min((k + 1) * 25000, NP_)),
                          close_pa, "pa", extra=pa_extra,
                          mid={SPLIT // SWBLK: trig_ag_a})

            # ---- AP sweep 0: accumulate catA contributions into partials ----
            def close_s0(j, c0, C, ps0):
                nc.scalar.activation(partial[:, c0:c0 + C], ps0[:, :C], AF.Identity)

            if STAGE >= 4:
                emit_pass(mS0, idxS0_h, dlS0_h,
                          lambda k: (catA_full, int(BOUND_S0[k]), int(BOUND_S0[k + 1])),
                          close_s0, "s0")

            # ---- AP sweep 1 + fused conv1/conv2/head epilogue ----
            def close_s1(j, c0, C, ps0):
                sum_t = sum_p.tile([128, SWBLK], f32, tag="sum")
                nc.vector.tensor_tensor(out=sum_t[:, :C], in0=ps0[:, :C],
                                        in1=partial[:, c0:c0 + C],
                                        op=mybir.AluOpType.add)
                inv_t = inv_p.tile([128, SWBLK], bf16, tag="inv128")
                nc.scalar.dma_start(inv_t[:, :C], invP_h[:, c0:c0 + C])
                meanA = mean_p.tile([64, SWBLK], bf16, tag="meanA")
                nc.vector.tensor_tensor(out=meanA[:, :C], in0=sum_t[0:64, :C],
                                        in1=inv_t[0:64, :C], op=mybir.AluOpType.mult)
                meanB = mean_p.tile([64, SWBLK], bf16, tag="meanB")
                nc.vector.tensor_tensor(out=meanB[:, :C], in0=sum_t[64:128, :C],
                                        in1=inv_t[64:128, :C], op=mybir.AluOpType.mult)
                pT_t = x65_p.tile([64, SWBLK], bf16, tag="pTt")
                nc.sync.dma_start(pT_t[:, :C], pT_h[:, c0:c0 + C])
                psE = psE_p.tile([64, SWBLK], f32, tag="psE")
                nc.tensor.matmul(psE[:, :C], wt["W1l_ap"][:], meanA[:, :C],
                                 start=True, stop=False)
                nc.tensor.matmul(psE[:, :C], wt["W1r_ap"][:], pT_t[:, :C],
                                 start=False, stop=True)
                p1 = big_p.tile([64, SWBLK], bf16, tag="big")
                nc.scalar.activation(p1[:, :C], psE[:, :C], AF.Relu, bias=wt["b1c"][:])
                psE2 = psE_p.tile([64, SWBLK], f32, tag="psE")
                nc.tensor.matmul(psE2[:, :C], wt["W2l_ap"][:], meanB[:, :C],
                                 start=True, stop=False)
                nc.tensor.matmul(psE2[:, :C], wt["W2r_ap"][:], p1[:, :C],
                                 start=False, stop=True)
                p2 = big_p.tile([64, SWBLK], bf16, tag="big")
                nc.scalar.activation(p2[:, :C], psE2[:, :C], AF.Relu, bias=wt["b2c"][:])
                psH = psE_p.tile([64, SWBLK], f32, tag="psE")
                nc.tensor.matmul(psH[:, :C], wt["Wl1"][:], p2[:, :C],
                                 start=True, stop=True)
                h = big_p.tile([64, SWBLK], bf16, tag="big")
                nc.scalar.activation(h[:, :C], psH[:, :C], AF.Relu, bias=wt["bl1c"][:])
                psO = psO_p.tile([1, SWBLK], f32, tag="psO")
                nc.tensor.matmul(psO[:, :C], wt["Wl2c"][:], h[:, :C],
                                 start=True, stop=True)
                o = outr_p.tile([1, SWBLK], f32, tag="outrow")
                nc.scalar.activation(o[:, :C], psO[:, :C], AF.Identity,
                                     bias=wt["bl2c"][:])
                nc.sync.dma_start(out_h[:, c0:c0 + C], o[:, :C])

            if STAGE >= 5:
                emit_pass(mS1, idxS1_h, dlS1_h,
                          lambda k: (catB_full, int(BOUND_S1[k]), int(BOUND_S1[k + 1])),
                          close_s1, "s1")

    nc.compile()

    if int(os.environ.get("KERNEL_TLSIM", "1")):
        try:
            from concourse.timeline_sim import TimelineSim
            _t = TimelineSim(nc)
            kernel.modeled_time_ns = _t.simulate()
            print(f"[kernel] TimelineSim modeled core time: "
                  f"{kernel.modeled_time_ns / 1e3:.1f} us")
        except Exception as e:
            print(f"[kernel] TimelineSim failed: {e}")
            kernel.modeled_time_ns = None

    globals()["_last_nc"] = nc
    if int(os.environ.get("KERNEL_BUILD_ONLY", "0")):
        raise SystemExit(0)

    in_maps = []
    for c in range(N_CORES):
        m = {"afT": afT_cores[c], "p_bf": p_bf, "pT": pT_cores[c],
             "invA": invA_cores[c], "invP": invP_cores[c],
             "idxPA": idxPA[c], "dlPA": dlPA[c],
             "idxS0": idxS0[c], "dlS0": dlS0[c],
             "idxS1": idxS1[c], "dlS1": dlS1[c],
             "ones": ones_bf, "iotaf": iota_stage}
        for n, v in zip(wb_names, wb_vals):
            m[n] = v
        for n, v in zip(wf_names, wf_vals):
            m[n] = v
        in_maps.append(m)

    if int(os.environ.get("KERNEL_SIM", "0")):
        from concourse import bass_interp
        sim = bass_interp.MultiCoreSim(nc, N_CORES)
        for c in range(N_CORES):
            for n, v in in_maps[c].items():
                sim.cores[c].tensor(n)[:] = v
        sim.simulate()
        results = [{"out": np.array(sim.cores[c].tensor("out"))} for c in range(N_CORES)]
    else:
        trace = bool(int(os.environ.get("KERNEL_TRACE", "0")))
        res = run_bass_kernel_spmd(nc, in_maps, core_ids=list(range(N_CORES)), trace=trace)
        if trace:
            kernel.last_exec_time_ns = res.exec_time_ns
            kernel.last_results = res
        results = res.results

    out = np.concatenate([results[c]["out"][0] for c in range(N_CORES)])[:NP_]
    return out.reshape(NP_, 1).astype(np.float32)
